# revision 11
# baseline (speedup 1.0000x reference)
"""Trainium2 Bass kernel v3 for the DNC memory-step problem (nn_DNC_3882650436261).

Pure data-parallel over batch (128 -> 16 items x 8 cores). Sort-free
allocation, link products via fp8 matmuls against host-prepped L / L^T,
mem_new never materialized. Hardware mapping highlights:

  - "m-part world": the 512-slot dim lives on partitions (4 chunks of 128);
    elementwise/softmax math runs at 128-lane width; slot sums = free-dim
    reduce over chunks + gpsimd partition_all_reduce.
  - allocation mask: u broadcast via PE ones-column matmuls, 64 bf16
    tensor_scalar is_gt compares (DVE), then the masked log-u reduction on
    the PE (cmp block stationary, lnu column streamed) landing m-part
    directly in one PSUM bank.  NB: PSUM accumulation groups must be
    emitted consecutively per output column or start=True data is lost.
  - phase B (mem products) uses memt chunks as the stationary operand so
    results come out slot-partitioned (no staging/transpose); phase F
    (read vectors) uses memn chunks stationary -> w-part results; the
    final combine stays w-part with one PE transpose per item-half.
  - all DMA'd tensors host-relaid so every transfer is contiguous per
    partition row; zero-dependency input loads are emitted before any
    dependent DMA (the sync issue queue is in-order, so a dependent DMA
    blocks every later issue).
  - work is split into item-halves (B/mask/C-head/link in one loop,
    cw/E/F/G in a second) so halves pipeline across engines; the five
    write-side scalars get an early partition broadcast, writers pinned
    to the vector engine to stay ordered with the softplus finalizing ws.

Self-contained: hardcodes all shapes; no file reads.
"""
import os
import numpy as np
import ml_dtypes
from contextlib import ExitStack

import concourse.bass as bass
import concourse.tile as tile
import concourse.bass_isa as bass_isa
from concourse import bacc, mybir
from concourse.bass_utils import run_bass_kernel_spmd

F32 = mybir.dt.float32
BF16 = mybir.dt.bfloat16
F32R = mybir.dt.float32r
FP8 = mybir.dt.float8e4
AF = mybir.ActivationFunctionType
OP = mybir.AluOpType
AX = mybir.AxisListType

B, M, W, R, IN = 128, 512, 128, 4, 1024
NCORES = 8
BL = B // NCORES            # 16 items per core
DELTA = 1e-6
NCH = M // 128              # 4 chunks of the slot dim
KIN = 1152                  # padded contraction dim (1024 + bias row -> 9*128)
DTOT = 919
NS = 34                     # per-item scalar table width
LSC = 512.0                 # fp8 pre-scale for link/rw
LDS = float(2.0 ** -18)     # rescale after fp8 link matmuls

_dims = dict(rk=R * W, rs=R, wk=W, ws=1, ev=W, wv=W, fg=R, ag=1, wg=1, rm=3 * R)
_ofs = {}
_o = 0
for _n, _d in _dims.items():
    _ofs[_n] = _o
    _o += _d
assert _o == DTOT

# scalar-table columns
S_FG, S_RS, S_WS, S_AG, S_WG = 0, 4, 8, 9, 10
S_M0, S_M1, S_M2 = 11, 15, 19
S_BNR, S_BW, S_RKWV, S_C3, S_OMAG = 23, 27, 28, 32, 33


def _emit(nc, tc, ctx, d):
    STOP = int(os.environ.get("KSTOP", "9"))

    P = ctx.enter_context(tc.tile_pool(name="persist", bufs=1))
    ps = ctx.enter_context(tc.tile_pool(name="ps", bufs=5, space=bass.MemorySpace.PSUM))
    psb = ctx.enter_context(tc.tile_pool(name="psb", bufs=2, space=bass.MemorySpace.PSUM))
    psm = ctx.enter_context(tc.tile_pool(name="psm", bufs=1, space=bass.MemorySpace.PSUM))
    stg = ctx.enter_context(tc.tile_pool(name="stg", bufs=2))
    memp = ctx.enter_context(tc.tile_pool(name="memp", bufs=8))
    mtp = ctx.enter_context(tc.tile_pool(name="mtp", bufs=8))
    lp = ctx.enter_context(tc.tile_pool(name="lp", bufs=10))
    scr = ctx.enter_context(tc.tile_pool(name="scr", bufs=8))
    mnp = ctx.enter_context(tc.tile_pool(name="mnp", bufs=8))

    def pst(pr, fr, pool=ps):
        return pool.tile([pr, fr], F32, tag="pst", name="pst")

    _uid = [0]

    def sct(fr=NCH * BL, dt=F32):
        # m-part scratch [128, fr] with a unique persistent allocation
        _uid[0] += 1
        return P.tile([128, fr], dt, tag=f"mp{_uid[0]}", name=f"mp{_uid[0]}")

    gates = P.tile([BL, DTOT], F32, tag="gates")

    def gsl(name, a=0, b=None):
        o = _ofs[name]
        if b is None:
            b = _dims[name]
        return gates[:, o + a:o + b]

    def _bail(lvl, t=None):
        if STOP <= lvl:
            if t is None:
                t = gates[:, 0:R * W]
            nc.sync.dma_start(d["out"][:], t)
            return True
        return False

    # view helpers (m-part free layouts)
    def vci(t):
        return t[:].rearrange("p (c i) -> p c i", i=BL)

    def vcir(t, k=R):
        return t[:].rearrange("p (c i k) -> p c i k", i=BL, k=k)

    # ---------------- constants + inputs resident in SBUF ----------------
    consts = P.tile([128, 129], F32, tag="consts")
    nc.sync.dma_start(consts[:], d["consts"][:])
    I128 = consts[:, 0:128]

    def ptrans(out_psum, in_sb):
        p = in_sb.shape[0]
        nc.tensor.transpose(out_psum, in_sb, I128[0:p, 0:p])

    xta = P.tile([128, 9, BL], BF16, tag="xta")
    nc.sync.dma_start(xta[:], d["xta"][:])
    wtf_sb = P.tile([128, 9, 6], BF16, tag="wtf")
    nc.sync.dma_start(wtf_sb[:], d["wtf"][:])



    # ---------------- phase A: fused linear + gates (item-part) ----------------
    zps = pst(BL, 512)
    zps2 = pst(BL, DTOT - 512)
    zpf = pst(BL, 6, psb)     # priority: fg/ag/wg columns (901:907)
    wp = ctx.enter_context(tc.tile_pool(name="wstream", bufs=9))
    wkts = []
    for k in range(9):
        nc.tensor.matmul(zpf[:], xta[:, k, :], wtf_sb[:, k, :],
                         start=(k == 0), stop=(k == 8))
    for k in range(9):
        wk_t = wp.tile([128, DTOT], BF16, tag="w")
        nc.sync.dma_start(wk_t[:], d["wta"][128 * k:128 * (k + 1), :])
        wkts.append(wk_t)

    RWT = P.tile([128, NCH * BL * R], F32, tag="RWT")    # rw^T [p,(c,i,r)]
    nc.sync.dma_start(RWT[:], d["rwt"][:])
    PRC = P.tile([128, NCH * BL], F32, tag="PRC")        # precedence^T
    nc.sync.dma_start(PRC[:], d["prct"][:])
    WWI = P.tile([128, NCH * BL], F32, tag="WWI")        # write_weights in ^T
    nc.sync.dma_start(WWI[:], d["wwt"][:])
    USG = P.tile([128, NCH * BL], F32, tag="USG")        # usage in ^T
    nc.sync.dma_start(USG[:], d["usgt"][:])
    DGT = P.tile([128, NCH * BL], F32, tag="DGT")        # diag(L) ^T
    nc.sync.dma_start(DGT[:], d["diagt"][:])

    # memory loads up front: zero-dependency DMAs must precede dependent ones
    # on the in-order sync issue queue
    mem2s, mn2s = [], []
    for j in range(BL // 2):
        mem2 = memp.tile([128, 2, M], BF16, tag="memt")
        nc.sync.dma_start(mem2[:], d["memt"][j])
        mem2s.append(mem2)
    for j in range(BL // 2):
        mn2 = mnp.tile([128, 2, NCH, W], BF16, tag="memn")
        nc.sync.dma_start(mn2[:], d["memn"][j])
        mn2s.append(mn2)

    def zsl(a, b):
        if b <= 512:
            return zps[:, a:b]
        assert a >= 512
        return zps2[:, a - 512:b - 512]

    nc.scalar.activation(gates[:, 901:907], zpf[:], AF.Sigmoid)           # fg,ag,wg (priority)
    fgrow = P.tile([1, BL * R], F32, tag="fgrow")
    nc.scalar.dma_start(fgrow[:], gsl("fg"))
    FGB = P.tile([128, BL * R], F32, tag="FGB")
    nc.gpsimd.partition_broadcast(FGB[:], fgrow[:])
    # ---------------- phase A2: usage / u / log u (m-part) ----------------
    fgb = FGB[:].rearrange("p (o i r) -> p o i r", o=1, r=R).broadcast_to([128, NCH, BL, R])
    psi4 = sct(NCH * BL * R)
    nc.vector.tensor_tensor(vcir(psi4), vcir(RWT), fgb, op=OP.mult)
    nc.any.tensor_scalar(psi4[:], psi4[:], 1.0, None, op0=OP.subtract)  # fg*rw - 1
    psi = sct()
    p4 = vcir(psi4)
    nc.vector.tensor_tensor(vci(psi), p4[:, :, :, 0], p4[:, :, :, 1], op=OP.mult)
    nc.vector.tensor_tensor(p4[:, :, :, 2], p4[:, :, :, 2], p4[:, :, :, 3], op=OP.mult)
    nc.vector.tensor_tensor(vci(psi), vci(psi), p4[:, :, :, 2], op=OP.mult)

    u_sb = P.tile([128, NCH * BL], F32, tag="u_sb")
    nc.vector.tensor_tensor(u_sb[:], USG[:], WWI[:], op=OP.mult)
    nc.vector.tensor_tensor(u_sb[:], USG[:], u_sb[:], op=OP.subtract)
    nc.vector.tensor_tensor(u_sb[:], u_sb[:], WWI[:], op=OP.add)
    nc.vector.tensor_tensor(u_sb[:], u_sb[:], psi[:], op=OP.mult)
    nc.any.tensor_scalar(u_sb[:], u_sb[:], 1.0 - DELTA, DELTA, op0=OP.mult, op1=OP.add)

    # u_ip (item-part copy of u) -> u_flat single row for mask broadcasts
    uips = pst(BL, M, psb)
    for c in range(NCH):
        ptrans(uips[:, 128 * c:128 * (c + 1)], vci(u_sb)[:, c, :])
    u_ip = P.tile([BL, M], F32, tag="u_ip")
    nc.any.tensor_copy(u_ip[:], uips[:])
    ub_ip = P.tile([BL, M], BF16, tag="ub_ip")
    nc.any.tensor_copy(ub_ip[:], uips[:])
    u_bf = P.tile([128, NCH * BL], BF16, tag="u_bf")
    nc.vector.tensor_copy(u_bf[:], u_sb[:])
    # one row tile per 4-item quarter so each UREPA broadcast matmul depends
    # on only its own two DMAs instead of all eight (whole-tile dependency)
    u_flatq = []
    for q in range(4):
        ufq = P.tile([1, 4 * M], BF16, tag=f"u_flat{q}", name=f"u_flat{q}")
        for j in range(2):
            nc.sync.dma_start(ufq[0:1, 1024 * j:1024 * (j + 1)],
                              ub_ip[4 * q + 2 * j:4 * q + 2 * j + 2, :])
        u_flatq.append(ufq)

    if _bail(2, u_ip[:, 0:R * W]):
        return

    for k in range(9):
        nc.tensor.matmul(zps[:], xta[:, k, :], wkts[k][:, 0:512],
                         start=(k == 0), stop=(k == 8))
        nc.tensor.matmul(zps2[:], xta[:, k, :], wkts[k][:, 512:DTOT],
                         start=(k == 0), stop=(k == 8))

    nc.scalar.activation(gates[:, 645:773], zsl(645, 773), AF.Sigmoid)           # ev
    nc.scalar.activation(gates[:, 0:512], zsl(0, 512), AF.Tanh)                  # rk
    nc.scalar.activation(gates[:, 516:644], zsl(516, 644), AF.Tanh)              # wk
    nc.scalar.activation(gates[:, 773:901], zsl(773, 901), AF.Tanh)              # wv
    nc.vector.tensor_copy(gates[:, 907:919], zsl(907, 919))                      # rm logits
    # softplus(z) = -ln(sigmoid(-z)): sigmoid rides the ev table load, one Ln load
    spts = {}
    for (a, b) in [(512, 516), (644, 645)]:
        spts[a] = scr.tile([BL, b - a], F32, tag="sp", name="spt")
        nc.scalar.activation(spts[a][:], zsl(a, b), AF.Sigmoid, scale=-1.0)
    for (a, b) in [(512, 516), (644, 645)]:
        nc.scalar.activation(spts[a][:], spts[a][:], AF.Ln)
    for (a, b) in [(512, 516), (644, 645)]:
        nc.vector.tensor_scalar(gates[:, a:b], spts[a][:], -1.0, None, op0=OP.mult)



    # read-mode softmax over groups of 3
    rmz = gates[:, 907:919].rearrange("i (r k) -> i r k", k=3)
    negmax3 = P.tile([BL, R], F32, tag="negmax3")
    nc.vector.tensor_reduce(negmax3[:], rmz, axis=AX.X, op=OP.max, negate=True)
    rme = P.tile([BL, 3 * R], F32, tag="rme")
    nc.vector.tensor_tensor(rme[:].rearrange("i (r k) -> i r k", k=3), rmz,
                            negmax3[:].rearrange("i (r o) -> i r o", o=1).broadcast_to([BL, R, 3]),
                            op=OP.add)
    nc.scalar.activation(rme[:], rme[:], AF.Exp)
    rmsum = P.tile([BL, R], F32, tag="rmsum")
    nc.vector.tensor_reduce(rmsum[:], rme[:].rearrange("i (r k) -> i r k", k=3), axis=AX.X, op=OP.add)
    nc.vector.reciprocal(rmsum[:], rmsum[:])
    rm = P.tile([BL, 3 * R], F32, tag="rm")
    nc.vector.tensor_tensor(rm[:].rearrange("i (r k) -> i r k", k=3),
                            rme[:].rearrange("i (r k) -> i r k", k=3),
                            rmsum[:].rearrange("i (r o) -> i r o", o=1).broadcast_to([BL, R, 3]),
                            op=OP.mult)

    # per-item key-norm scalars (item-part)
    scw = P.tile([BL, W], F32, tag="scw")
    bw128 = P.tile([BL, 1], F32, tag="bw128")
    nc.scalar.activation(scw[:], gsl("wk"), AF.Square, accum_out=bw128[:])
    nc.scalar.activation(bw128[:], bw128[:], AF.Sqrt)
    nc.any.tensor_scalar(bw128[:], bw128[:], float(W), float(W) * DELTA, op0=OP.mult, op1=OP.add)

    # early broadcast of the 5 write-side scalars the C-head needs (the full
    # SCB table waits on read-side norms which are only needed by phase E)
    NSW = 5
    W_WS, W_AG, W_WG, W_BW, W_OMAG = 0, 1, 2, 3, 4
    # all writers pinned to vector so they are in-order with the softplus add
    # that finalizes gates["ws"] (cross-engine timing there proved racy)
    SCALW = P.tile([BL, NSW], F32, tag="SCALW")
    nc.vector.tensor_copy(SCALW[:, W_WS:W_WS + 1], gsl("ws"))
    nc.vector.tensor_copy(SCALW[:, W_AG:W_AG + 1], gsl("ag"))
    nc.vector.tensor_copy(SCALW[:, W_WG:W_WG + 1], gsl("wg"))
    nc.vector.tensor_copy(SCALW[:, W_BW:W_BW + 1], bw128[:])
    nc.vector.tensor_scalar(SCALW[:, W_OMAG:W_OMAG + 1], gsl("ag"), -1.0, 1.0,
                            op0=OP.mult, op1=OP.add)
    scalwrow = P.tile([1, BL * NSW], F32, tag="scalwrow")
    nc.sync.dma_start(scalwrow[:], SCALW[:])
    SCW = P.tile([128, BL * NSW], F32, tag="SCW")
    nc.gpsimd.partition_broadcast(SCW[:], scalwrow[:])

    def scw_c(s):
        return SCW[:].rearrange("p (o i s) -> p o i s", o=1, s=NSW)[:, :, :, s] \
            .broadcast_to([128, NCH, BL])

    bnr = P.tile([BL, R], F32, tag="bnr")
    rkwv = P.tile([BL, R], F32, tag="rkwv")
    for r in range(R):
        nc.scalar.activation(scw[:], gsl("rk", r * W, (r + 1) * W), AF.Square, accum_out=bnr[:, r:r + 1])
        nc.vector.tensor_tensor(scw[:], gsl("rk", r * W, (r + 1) * W), gsl("wv"), op=OP.mult)
        nc.vector.tensor_scalar(scw[:], scw[:], 1.0, None, op0=OP.mult, op1=OP.add,
                                accum_out=rkwv[:, r:r + 1])
    nc.scalar.activation(bnr[:], bnr[:], AF.Sqrt)
    nc.any.tensor_scalar(bnr[:], bnr[:], float(W), float(W) * DELTA, op0=OP.mult, op1=OP.add)
    c3 = P.tile([BL, 1], F32, tag="c3")
    nc.scalar.activation(scw[:], gsl("wv"), AF.Square, accum_out=c3[:])

    # KCM/NRM lhsT tables (w on partitions), f32 then cast to bf16
    KCMf = P.tile([128, BL * 11], F32, tag="KCMf")
    NRMf = P.tile([128, BL * 3], F32, tag="NRMf")
    EVT = P.tile([128, BL], F32, tag="EVT")

    def kcm_col(j):
        return KCMf[:].rearrange("p (i k) -> p i k", k=11)[:, :, j]

    gtp = pst(128, BL, psb)
    ptrans(gtp[:], gsl("wk"))
    nc.any.tensor_copy(kcm_col(0), gtp[:])
    gtp = pst(128, BL, psb)
    ptrans(gtp[:], gsl("ev"))
    nc.any.tensor_copy(EVT[:], gtp[:])
    gtp = pst(128, BL, psb)
    ptrans(gtp[:], gsl("wv"))
    nc.any.tensor_copy(kcm_col(9), gtp[:])
    nc.vector.tensor_tensor(kcm_col(10), kcm_col(9), EVT[:], op=OP.mult)  # ev*wv
    for r in range(R):
        gtp = pst(128, BL, psb)
        ptrans(gtp[:], gsl("rk", r * W, (r + 1) * W))
        nc.any.tensor_copy(kcm_col(1 + r), gtp[:])
        nc.vector.tensor_tensor(kcm_col(5 + r), kcm_col(1 + r), EVT[:], op=OP.mult)
    nrm3 = NRMf[:].rearrange("p (i k) -> p i k", k=3)
    nc.any.memset(nrm3[:, :, 0], 1.0)
    nc.any.tensor_copy(nrm3[:, :, 1], EVT[:])
    nc.scalar.activation(nrm3[:, :, 2], EVT[:], AF.Square)
    KCMb = P.tile([128, BL * 11], BF16, tag="KCMb")
    nc.any.tensor_copy(KCMb[:], KCMf[:])
    NRMb = P.tile([128, BL * 3], BF16, tag="NRMb")
    nc.any.tensor_copy(NRMb[:], NRMf[:])

    # ---------------- scalar table -> partition-broadcast SCB ----------------
    SCAL = P.tile([BL, NS], F32, tag="SCAL")
    nc.any.tensor_copy(SCAL[:, S_FG:S_FG + R], gsl("fg"))
    nc.any.tensor_copy(SCAL[:, S_RS:S_RS + R], gsl("rs"))
    nc.any.tensor_copy(SCAL[:, S_WS:S_WS + 1], gsl("ws"))
    nc.any.tensor_copy(SCAL[:, S_AG:S_AG + 1], gsl("ag"))
    nc.any.tensor_copy(SCAL[:, S_WG:S_WG + 1], gsl("wg"))
    for k in range(3):
        nc.any.tensor_copy(SCAL[:, S_M0 + R * k:S_M0 + R * (k + 1)],
                           rm[:].rearrange("i (r k) -> i r k", k=3)[:, :, k])
    nc.any.tensor_copy(SCAL[:, S_BNR:S_BNR + R], bnr[:])
    nc.any.tensor_copy(SCAL[:, S_BW:S_BW + 1], bw128[:])
    nc.any.tensor_copy(SCAL[:, S_RKWV:S_RKWV + R], rkwv[:])
    nc.any.tensor_copy(SCAL[:, S_C3:S_C3 + 1], c3[:])
    nc.any.tensor_scalar(SCAL[:, S_OMAG:S_OMAG + 1], gsl("ag"), -1.0, 1.0,
                         op0=OP.mult, op1=OP.add)
    scalrow = P.tile([1, BL * NS], F32, tag="scalrow")
    nc.scalar.dma_start(scalrow[:], SCAL[:])
    SCB = P.tile([128, BL * NS], F32, tag="SCB")
    nc.gpsimd.partition_broadcast(SCB[:], scalrow[:])

    def scb(s, w=1):
        # [128, BL, w] view of scalar cols s..s+w
        return SCB[:].rearrange("p (i s) -> p i s", s=NS)[:, :, s:s + w]

    def scb_c(s):
        # broadcast over chunks -> [128, NCH, BL]
        return SCB[:].rearrange("p (o i s) -> p o i s", o=1, s=NS)[:, :, :, s] \
            .broadcast_to([128, NCH, BL])

    def scb_cr(s):
        # per-(i,r) scalars broadcast over chunks -> [128, NCH, BL, R]
        return SCB[:].rearrange("p (o i s) -> p o i s", o=1, s=NS)[:, :, :, s:s + R] \
            .broadcast_to([128, NCH, BL, R])

    def bc_r(t):
        # [128, NCH*BL] -> [128, NCH, BL, R] broadcast over r
        return t[:].rearrange("p (c i o) -> p c i o", i=BL, o=1).broadcast_to([128, NCH, BL, R])

    # ---------------- allocation mask v2 (ts compare + PE reduce) ----------------
    # Emitted AFTER the SCAL/SCB section so the 64 is_gt ops don't starve the
    # vector-engine ops feeding the scalar-table broadcast (C-head dependency).
    # S_acc[p,(c,i)] = sum_j [u_i[j] < u_i[slot(c,p)]] * ln u_i[j], computed as:
    #   cmp_t[j, p'] = (u[p'] > u[j])  (tensor_scalar is_gt, bf16)
    #   S col (cp,i) = sum_{cj} cmp_t_block^T @ lnu_col   (PE, m-part direct)
    # broadcast u_flat across partitions via PE ones-column matmuls (PE is idle
    # here; the serial gpsimd broadcasts were 3.2us each on the critical path)
    ones_row = P.tile([1, 128], BF16, tag="ones_row")
    nc.vector.memset(ones_row[:], 1.0)
    NQ = 4
    IQ = BL // NQ
    urepq = []
    for q in range(NQ):
        uq = P.tile([128, IQ * M], BF16, tag=f"UREPA{q}", name=f"UREPA{q}")
        for s in range(IQ):
            i = IQ * q + s
            ubx = pst(128, M, psb)
            nc.tensor.matmul(ubx[:], ones_row[:],
                             u_flatq[i // 4][0:1, M * (i % 4):M * (i % 4 + 1)],
                             start=True, stop=True)
            if i % 2 == 0:
                nc.scalar.activation(uq[:, M * s:M * (s + 1)], ubx[:], AF.Copy)
            else:
                nc.vector.tensor_copy(uq[:, M * s:M * (s + 1)], ubx[:])
        urepq.append(uq)
    LNU = P.tile([128, NCH * BL], BF16, tag="LNU")
    nc.scalar.activation(LNU[:], u_sb[:], AF.Ln)
    # u rounded to bf16 then held in f32, so the is_gt scalar sees the same
    # rounding as the bf16 in0 (a slot must not compare unequal to itself)
    u_bf32 = P.tile([128, NCH * BL], F32, tag="u_bf32")
    nc.vector.tensor_copy(u_bf32[:], u_bf[:])
    S_PS = psm.tile([128, NCH * BL], F32, tag="sps", name="sps")
    sps = S_PS[:].rearrange("p (c i) -> p c i", i=BL)

    def emit_mask_items(items):
        for i in items:
            cts = []
            for cj in range(NCH):
                cmp_t = scr.tile([128, M], BF16, tag="mscr", name="mscr")
                nc.vector.tensor_scalar(cmp_t[:], urepq[i // IQ][:, M * (i % IQ):M * (i % IQ + 1)],
                                        vci(u_bf32)[:, cj, i:i + 1], None, op0=OP.is_gt)
                cts.append(cmp_t)
            for cp in range(NCH):
                for cj in range(NCH):
                    nc.tensor.matmul(sps[:, cp, i:i + 1], cts[cj][:, 128 * cp:128 * (cp + 1)],
                                     LNU[:, cj * BL + i:cj * BL + i + 1],
                                     start=(cj == 0), stop=(cj == NCH - 1))

    if _bail(1):
        return

    # ---------------- phase B: memory products (per item, m-part out) ----------------
    # out[m-chunk, col] = sum_w memt[w, m]*KCM[w, col]  (and mt2 for norms):
    # the matmul produces slot-partitioned results directly; one small strided
    # PSUM->SBUF copy per item replaces the old stage+dma_transpose pipeline.
    NB = 14
    BCOLL = P.tile([128, NCH * BL * NB], BF16, tag="BCOLL")
    mt2s = []
    for i in range(BL):
        mt2 = mtp.tile([128, M], BF16, tag="mt2")
        nc.scalar.activation(mt2[:], mem2s[i // 2][:, i % 2, :], AF.Square)
        mt2s.append(mt2)
    def emit_B_items(items):
        for i in items:
            mti = mem2s[i // 2][:, i % 2, :]
            bp = pst(128, NCH * NB)
            for c in range(NCH):
                nc.tensor.matmul(bp[:, NB * c:NB * c + 11], mti[:, 128 * c:128 * (c + 1)],
                                 KCMb[:, 11 * i:11 * (i + 1)], start=True, stop=True)
                nc.tensor.matmul(bp[:, NB * c + 11:NB * c + 14],
                                 mt2s[i][:, 128 * c:128 * (c + 1)],
                                 NRMb[:, 3 * i:3 * (i + 1)], start=True, stop=True)
            dst = BCOLL[:].rearrange("p (c i k) -> p i c k", i=BL, k=NB)[:, i, :, :]
            nc.scalar.activation(dst, bp[:].rearrange("p (c k) -> p c k", k=NB), AF.Copy)

    # ---------------- phase C head: alloc / wcw / ww / fp8 lhsT ----------------
    bcf = vcir(BCOLL, NB)

    def bcol(j):
        return bcf[:, :, :, j]

    WCN, T1, T2, S0, S1, S2 = bcol(0), bcol(9), bcol(10), bcol(11), bcol(12), bcol(13)

    EXS, onemu, alloc = sct(), sct(), sct()
    AO, wden, wz, wcw = sct(), sct(), sct(), sct()
    wzs = P.tile([128, BL], F32, tag="wzs")
    WZS = P.tile([128, BL], F32, tag="WZS")
    ww = P.tile([128, NCH * BL], F32, tag="ww")
    RWC8 = P.tile([128, NCH * BL * 8], FP8, tag="RWC8")
    rwc = vcir(RWC8, 8)
    DCOLL = P.tile([128, NCH * BL * 64], BF16, tag="DCOLL")

    for h in range(2):
        sl = slice(8 * h, 8 * h + 8)
        # --- phase-B half first so its scalar copies precede this half's
        #     C-head ops in the in-order scalar queue ---
        emit_B_items(range(8 * h, 8 * h + 8))
        # --- allocation-mask half (is_gt + PE reduce for these items) ---
        emit_mask_items(range(8 * h, 8 * h + 8))
        # --- C-head for this half ---
        nc.scalar.activation(vci(EXS)[:, :, sl], sps[:, :, sl], AF.Exp)
        nc.vector.tensor_scalar(vci(onemu)[:, :, sl], vci(u_sb)[:, :, sl], -1.0, 1.0,
                                op0=OP.mult, op1=OP.add)
        nc.vector.tensor_tensor(vci(alloc)[:, :, sl], vci(onemu)[:, :, sl],
                                vci(EXS)[:, :, sl], op=OP.mult)
        nc.scalar.activation(vci(AO)[:, :, sl], S0[:, :, sl], AF.Sqrt)
        nc.vector.tensor_scalar(vci(AO)[:, :, sl], vci(AO)[:, :, sl], 1.0, DELTA,
                                op0=OP.mult, op1=OP.add)
        nc.vector.tensor_tensor(vci(wden)[:, :, sl], vci(AO)[:, :, sl],
                                scw_c(W_BW)[:, :, sl], op=OP.mult)
        nc.vector.tensor_scalar(vci(wden)[:, :, sl], vci(wden)[:, :, sl], 1.0, DELTA,
                                op0=OP.mult, op1=OP.add)
        nc.vector.reciprocal(vci(wden)[:, :, sl], vci(wden)[:, :, sl])
        nc.vector.tensor_tensor(vci(wz)[:, :, sl], WCN[:, :, sl], vci(wden)[:, :, sl],
                                op=OP.mult)
        nc.vector.tensor_tensor(vci(wz)[:, :, sl], vci(wz)[:, :, sl],
                                scw_c(W_WS)[:, :, sl], op=OP.mult)
        nc.scalar.activation(vci(wz)[:, :, sl], vci(wz)[:, :, sl], AF.Exp)
        nc.vector.tensor_reduce(wzs[:, sl], wz[:].rearrange("p (c i) -> p i c", i=BL)[:, sl, :],
                                axis=AX.X, op=OP.add)
        nc.gpsimd.partition_all_reduce(WZS[:, sl], wzs[:, sl], channels=128,
                                       reduce_op=bass_isa.ReduceOp.add)
        nc.vector.reciprocal(WZS[:, sl], WZS[:, sl])
        nc.vector.tensor_tensor(vci(wcw)[:, :, sl], vci(wz)[:, :, sl],
                                WZS[:].rearrange("p (o i) -> p o i", o=1)[:, :, sl]
                                .broadcast_to([128, NCH, 8]), op=OP.mult)
        nc.vector.tensor_tensor(vci(alloc)[:, :, sl], vci(alloc)[:, :, sl],
                                scw_c(W_AG)[:, :, sl], op=OP.mult)
        nc.vector.tensor_tensor(vci(ww)[:, :, sl], vci(wcw)[:, :, sl],
                                scw_c(W_OMAG)[:, :, sl], op=OP.mult)
        nc.vector.tensor_tensor(vci(ww)[:, :, sl], vci(ww)[:, :, sl],
                                vci(alloc)[:, :, sl], op=OP.add)
        nc.vector.tensor_tensor(vci(ww)[:, :, sl], vci(ww)[:, :, sl],
                                scw_c(W_WG)[:, :, sl], op=OP.mult)
        for c in range(NCH):
            nc.vector.tensor_scalar(rwc[:, c, sl, 0:4], vcir(RWT)[:, c, sl, :], LSC, None,
                                    op0=OP.mult)
            nc.vector.scalar_tensor_tensor(rwc[:, c, sl, 4:8], vcir(RWT)[:, c, sl, :], LSC,
                                           bc_r(ww)[:, c, sl, :], op0=OP.mult, op1=OP.mult)
        # --- D for this half ---
        stg2d = None
        for i in range(8 * h, 8 * h + 8):
            ll = lp.tile([128, 2, NCH, M], FP8, tag="ll")
            nc.sync.dma_start(ll[:], d["llt"][i])
            if i % 2 == 0:
                stg2d = stg.tile([128, M], BF16, tag="stg2", name="stgD")
            bps = pst(8, M)
            fps = pst(8, M)
            for cp in range(NCH // 2):
                lhs2 = RWC8[:].rearrange("p (c i k) -> p c i k", i=BL, k=8)[:, 2 * cp:2 * cp + 2, i, :]
                nc.tensor.matmul(bps[:], lhs2, ll[:, 0, 2 * cp:2 * cp + 2, :],
                                 start=(cp == 0), stop=(cp == 1),
                                 perf_mode=mybir.MatmulPerfMode.DoubleRow)
                nc.tensor.matmul(fps[:], lhs2, ll[:, 1, 2 * cp:2 * cp + 2, :],
                                 start=(cp == 0), stop=(cp == 1),
                                 perf_mode=mybir.MatmulPerfMode.DoubleRow)
            o = 64 * (i % 2)
            nc.scalar.activation(stg2d[o:o + 8, :], bps[:], AF.Copy, scale=LDS)
            nc.vector.tensor_scalar(stg2d[o + 32:o + 40, :], fps[:], LDS, None, op0=OP.mult)
            if i % 2 == 1:
                dst = DCOLL[:].rearrange("p (c i k) -> p c (i k)", i=BL, k=64)[:, :, 64 * (i - 1):64 * (i + 1)]
                nc.sync.dma_start_transpose(dst, stg2d[:])

    if _bail(6, u_ip[:, 0:R * W]):
        return





    # ---------------- phases E+F per item-half (pipelined with D) ----------------
    BH = 8
    dcv = vcir(DCOLL, 64)
    out_sb = P.tile([BL * R, W], F32, tag="out_sb")   # row (i*R+r) = read vector
    RVL = P.tile([128, NCH * BL * 8], BF16, tag="RVL")
    rvv = vcir(RVL, 8)


    def sctH(fr=NCH * BH, dt=F32):
        _uid[0] += 1
        return P.tile([128, fr], dt, tag=f"mh{_uid[0]}", name=f"mh{_uid[0]}")

    def vciH(t):
        return t[:].rearrange("p (c i) -> p c i", i=BH)

    def vcirH(t, k=R):
        return t[:].rearrange("p (c i k) -> p c i k", i=BH, k=k)

    def mk_bcH(sl):
        def bcH(x):
            # slice of full m-part [128, NCH*BL] -> bcast [128, NCH, BH, R]
            return x[:].rearrange("p (c i o) -> p c i o", i=BL, o=1)[:, :, sl, :] \
                .broadcast_to([128, NCH, BH, R])
        return bcH

    def bcHt(t):
        # per-half tile [128, NCH*BH] -> bcast over r
        return t[:].rearrange("p (c i o) -> p c i o", i=BH, o=1) \
            .broadcast_to([128, NCH, BH, R])

    cwm2s = []
    for h in range(2):
        sl = slice(BH * h, BH * (h + 1))
        bcH = mk_bcH(sl)
        wwH = vci(ww)[:, :, sl]
        S0h, S1h, S2h = S0[:, :, sl], S1[:, :, sl], S2[:, :, sl]
        T1h, T2h = T1[:, :, sl], T2[:, :, sl]

        # new-memory norms AN
        ww2 = sctH()
        nc.vector.tensor_tensor(vciH(ww2), wwH, wwH, op=OP.mult)
        q1 = sctH()
        nc.vector.tensor_tensor(vciH(q1), S1h, T1h, op=OP.subtract)
        nc.vector.scalar_tensor_tensor(vciH(q1), wwH, -2.0, vciH(q1), op0=OP.mult, op1=OP.mult)
        q2 = sctH()
        nc.vector.scalar_tensor_tensor(vciH(q2), T2h, -2.0, S2h, op0=OP.mult, op1=OP.add)
        nc.vector.tensor_tensor(vciH(q2), vciH(q2), scb_c(S_C3)[:, :, sl], op=OP.add)
        nc.vector.tensor_tensor(q2[:], q2[:], ww2[:], op=OP.mult)
        AN = sctH()
        nc.vector.tensor_tensor(vciH(AN), S0h, vciH(q1), op=OP.add)
        nc.vector.tensor_tensor(AN[:], AN[:], q2[:], op=OP.add)
        nc.scalar.activation(AN[:], AN[:], AF.Sqrt)
        nc.any.tensor_scalar(AN[:], AN[:], 1.0, DELTA, op0=OP.mult, op1=OP.add)

        # read content weights cw (scaled by mode2 / csum)
        cnum = sctH(NCH * BH * R)
        cn = vcirH(cnum)
        cwA = bcf[:, :, sl, 1:5]
        cwB = bcf[:, :, sl, 5:9]
        nc.vector.tensor_tensor(cn, cwB, bcH(ww), op=OP.mult)
        nc.vector.tensor_tensor(cn, cwA, cn, op=OP.subtract)
        ct = sctH(NCH * BH * R)
        nc.vector.tensor_tensor(vcirH(ct), bcH(ww), scb_cr(S_RKWV)[:, :, sl, :], op=OP.mult)
        nc.vector.tensor_tensor(cnum[:], cnum[:], ct[:], op=OP.add)
        cden = sctH(NCH * BH * R)
        nc.vector.tensor_tensor(vcirH(cden), bcHt(AN), scb_cr(S_BNR)[:, :, sl, :], op=OP.mult)
        nc.any.tensor_scalar(cden[:], cden[:], 1.0, DELTA, op0=OP.mult, op1=OP.add)
        nc.vector.reciprocal(cden[:], cden[:])
        nc.vector.tensor_tensor(cnum[:], cnum[:], cden[:], op=OP.mult)
        nc.vector.tensor_tensor(cn, cn, scb_cr(S_RS)[:, :, sl, :], op=OP.mult)
        nc.scalar.activation(cnum[:], cnum[:], AF.Exp)
        csum = sctH(BH * R)
        nc.vector.tensor_reduce(csum[:], cnum[:].rearrange("p (c j) -> p j c", j=BH * R),
                                axis=AX.X, op=OP.add)
        CSR = sctH(BH * R)
        nc.gpsimd.partition_all_reduce(CSR[:], csum[:], channels=128,
                                       reduce_op=bass_isa.ReduceOp.add)
        nc.vector.reciprocal(CSR[:], CSR[:])
        nc.vector.tensor_tensor(CSR[:].rearrange("p (i r) -> p i r", r=R),
                                CSR[:].rearrange("p (i r) -> p i r", r=R),
                                scb(S_M2, R)[:, sl, :], op=OP.mult)
        cwm2 = sctH(NCH * BH * R)   # mode2 * cw
        nc.vector.tensor_tensor(vcirH(cwm2), cn,
                                CSR[:].rearrange("p (o i r) -> p o i r", o=1, r=R)
                                .broadcast_to([128, NCH, BH, R]), op=OP.mult)
        cwm2s.append(cwm2)

    for h in range(2):
        sl = slice(BH * h, BH * (h + 1))
        bcH = mk_bcH(sl)
        wwH = vci(ww)[:, :, sl]
        cwm2 = cwm2s[h]

        # ---- phase E: assemble fwd/bwd/rw_new (m-part) ----
        P1, P2 = dcv[:, :, sl, 0:4], dcv[:, :, sl, 4:8]
        F1, F2 = dcv[:, :, sl, 32:36], dcv[:, :, sl, 36:40]
        rwtH = vcir(RWT)[:, :, sl, :]
        prcbH = PRC[:].rearrange("p (c i o) -> p c i o", i=BL, o=1)[:, :, sl, :] \
            .broadcast_to([128, NCH, BH, R])

        # cpr = prec . rw_r ; dwr = rw_r . ww   (per item, read head)
        scr4 = sctH(NCH * BH * R)
        nc.vector.tensor_tensor(vcirH(scr4), rwtH, prcbH, op=OP.mult)
        CDW = sctH(2 * BH * R)
        nc.vector.tensor_reduce(CDW[:, 0:BH * R],
                                scr4[:].rearrange("p (c j) -> p j c", j=BH * R),
                                axis=AX.X, op=OP.add)
        scr4b = sctH(NCH * BH * R)
        nc.vector.tensor_tensor(vcirH(scr4b), rwtH, bcH(ww), op=OP.mult)
        nc.vector.tensor_reduce(CDW[:, BH * R:2 * BH * R],
                                scr4b[:].rearrange("p (c j) -> p j c", j=BH * R),
                                axis=AX.X, op=OP.add)
        CDWr = sctH(2 * BH * R)
        nc.gpsimd.partition_all_reduce(CDWr[:], CDW[:], channels=128,
                                       reduce_op=bass_isa.ReduceOp.add)

        def cdw_b(off):
            return CDWr[:, off:off + BH * R].rearrange("p (o i r) -> p o i r", o=1, r=R) \
                .broadcast_to([128, NCH, BH, R])

        # dv = (1-2ww)*diag + ww*prec ; DR = rw * dv
        dv = sctH()
        nc.vector.tensor_scalar(vciH(dv), wwH, -2.0, 1.0, op0=OP.mult, op1=OP.add)
        nc.vector.tensor_tensor(vciH(dv), vciH(dv), vci(DGT)[:, :, sl], op=OP.mult)
        t2m = sctH()
        nc.vector.tensor_tensor(vciH(t2m), wwH, vci(PRC)[:, :, sl], op=OP.mult)
        nc.vector.tensor_tensor(dv[:], dv[:], t2m[:], op=OP.add)
        DR = sctH(NCH * BH * R)
        nc.vector.tensor_tensor(vcirH(DR), rwtH, bcHt(dv), op=OP.mult)

        onemw = sctH()
        nc.vector.tensor_scalar(vciH(onemw), wwH, -1.0, 1.0, op0=OP.mult, op1=OP.add)

        # fwd = F1*(1-ww) - F2 + ww (x) cpr - DR   (then scaled by mode1)
        fwd = sctH(NCH * BH * R)
        fv = vcirH(fwd)
        nc.vector.tensor_tensor(fv, F1, bcHt(onemw), op=OP.mult)
        nc.vector.tensor_tensor(fv, fv, F2, op=OP.subtract)
        ftt = sctH(NCH * BH * R)
        nc.vector.tensor_tensor(vcirH(ftt), bcH(ww), cdw_b(0), op=OP.mult)
        nc.vector.tensor_tensor(fwd[:], fwd[:], ftt[:], op=OP.add)
        nc.vector.tensor_tensor(fwd[:], fwd[:], DR[:], op=OP.subtract)

        # bwd = P1*(1-ww) - P2 + prec (x) dwr - DR  (then scaled by mode0)
        bwd = sctH(NCH * BH * R)
        bv = vcirH(bwd)
        nc.vector.tensor_tensor(bv, P1, bcHt(onemw), op=OP.mult)
        nc.vector.tensor_tensor(bv, bv, P2, op=OP.subtract)
        nc.vector.tensor_tensor(vcirH(ftt), prcbH, cdw_b(BH * R), op=OP.mult)
        nc.vector.tensor_tensor(bwd[:], bwd[:], ftt[:], op=OP.add)
        nc.vector.tensor_tensor(bwd[:], bwd[:], DR[:], op=OP.subtract)

        rwnew = sctH(NCH * BH * R)
        nc.vector.tensor_tensor(bv, bv, scb_cr(S_M0)[:, :, sl, :], op=OP.mult)
        nc.vector.tensor_tensor(fv, fv, scb_cr(S_M1)[:, :, sl, :], op=OP.mult)
        nc.vector.tensor_tensor(rwnew[:], bwd[:], fwd[:], op=OP.add)
        nc.vector.tensor_tensor(rwnew[:], rwnew[:], cwm2[:], op=OP.add)

        # sc = rwnew . ww
        nc.vector.tensor_tensor(vcirH(scr4), vcirH(rwnew), bcH(ww), op=OP.mult)
        SC1 = sctH(BH * R)
        nc.vector.tensor_reduce(SC1[:], scr4[:].rearrange("p (c j) -> p j c", j=BH * R),
                                axis=AX.X, op=OP.add)
        SCR_ = sctH(BH * R)
        nc.gpsimd.partition_all_reduce(SCR_[:], SC1[:], channels=128,
                                       reduce_op=bass_isa.ReduceOp.add)

        # bf16 lhsT for read vectors: [rwnew | rwnew*ww]
        nc.any.tensor_copy(rvv[:, :, sl, 0:4], vcirH(rwnew))
        nc.vector.tensor_tensor(rvv[:, :, sl, 4:8], vcirH(rwnew), bcH(ww), op=OP.mult)

        # ---- phase F: read vectors, w-part out (lhsT = memn chunk) ----
        # trp[w, k] = sum_m mem[m, w] * rvl[m, k]; k = [rw_new heads | rw_new*ww heads]
        TRH = sctH(BH * 8)
        for i in range(BH * h, BH * (h + 1)):
            trp = pst(128, 8)
            for c in range(NCH):
                nc.tensor.matmul(trp[:], mn2s[i // 2][:, i % 2, c, :], rvv[:, c, i, :],
                                 start=(c == 0), stop=(c == NCH - 1))
            nc.vector.tensor_copy(TRH[:, 8 * (i - BH * h):8 * (i - BH * h) + 8], trp[:])

        # ---- phase G: final combine (w-part), then PE transpose to item rows ----
        trv = TRH[:].rearrange("p (i k) -> p i k", k=8)
        TRA, TRB = trv[:, :, 0:4], trv[:, :, 4:8]
        evb = EVT[:].rearrange("p (i o) -> p i o", o=1)[:, sl, :].broadcast_to([128, BH, R])
        wvb = KCMf[:].rearrange("p (i k) -> p i k", k=11)[:, sl, 9:10].broadcast_to([128, BH, R])
        og = sctH(BH * R)
        ogv = og[:].rearrange("p (i r) -> p i r", r=R)
        nc.vector.tensor_tensor(ogv, TRB, evb, op=OP.mult)
        nc.vector.tensor_tensor(ogv, TRA, ogv, op=OP.subtract)
        og2 = sctH(BH * R)
        nc.vector.tensor_tensor(og2[:].rearrange("p (i r) -> p i r", r=R),
                                wvb, SCR_[:].rearrange("p (i r) -> p i r", r=R), op=OP.mult)
        nc.vector.tensor_tensor(og[:], og[:], og2[:], op=OP.add)
        otp = pst(BH * R, 128, psb)
        ptrans(otp[:], og[:])
        nc.vector.tensor_copy(out_sb[32 * h:32 * (h + 1), :], otp[:])
        nc.sync.dma_start(d["out"][BH * h:BH * (h + 1), :],
                          out_sb[32 * h:32 * (h + 1), :])


_NC_CACHE = {}


def build_nc():
    if "nc" in _NC_CACHE:
        return _NC_CACHE["nc"]
    nc = bacc.Bacc("TRN2", target_bir_lowering=False, debug=False)
    d = {}
    d["consts"] = nc.dram_tensor("consts", [128, 129], F32, kind="ExternalInput")
    d["xta"] = nc.dram_tensor("xta", [128, KIN // 128, BL], BF16, kind="ExternalInput")
    d["wta"] = nc.dram_tensor("wta", [KIN, DTOT], BF16, kind="ExternalInput")
    d["wtf"] = nc.dram_tensor("wtf", [128, KIN // 128, 6], BF16, kind="ExternalInput")
    d["memt"] = nc.dram_tensor("memt", [BL // 2, W, 2, M], BF16, kind="ExternalInput")
    d["memn"] = nc.dram_tensor("memn", [BL // 2, 128, 2, M // 128, W], BF16, kind="ExternalInput")
    d["llt"] = nc.dram_tensor("llt", [BL, 128, 2, M // 128, M], FP8, kind="ExternalInput")
    d["rwt"] = nc.dram_tensor("rwt", [128, M // 128, BL * R], F32, kind="ExternalInput")
    d["prct"] = nc.dram_tensor("prct", [128, M // 128, BL], F32, kind="ExternalInput")
    d["wwt"] = nc.dram_tensor("wwt", [128, M // 128, BL], F32, kind="ExternalInput")
    d["usgt"] = nc.dram_tensor("usgt", [128, M // 128, BL], F32, kind="ExternalInput")
    d["diagt"] = nc.dram_tensor("diagt", [128, M // 128, BL], F32, kind="ExternalInput")
    d["out"] = nc.dram_tensor("out", [BL, R * W], F32, kind="ExternalOutput")
    with tile.TileContext(nc) as tc:
        with ExitStack() as ctx:
            _emit(nc, tc, ctx, d)
    nc.compile()
    _NC_CACHE["nc"] = nc
    return nc


def make_in_maps(inputs):
    names = ["rk", "rs", "wk", "ws", "ev", "wv", "fg", "ag", "wg", "rm"]
    Wall = np.concatenate([np.asarray(inputs[f"W_{n}"]) for n in names], axis=0).astype(np.float32)
    ball = np.concatenate([np.asarray(inputs[f"b_{n}"]) for n in names], axis=0).astype(np.float32)
    wta = np.zeros((KIN, DTOT), np.float32)
    wta[:IN] = Wall.T
    wta[IN] = ball
    wtf = np.ascontiguousarray(wta[:, 901:907]).astype(ml_dtypes.bfloat16)
    wta = wta.astype(ml_dtypes.bfloat16)
    consts = np.zeros((128, 129), np.float32)
    consts[:, :128] = np.eye(128, dtype=np.float32)
    consts[:, 128] = 1.0

    x = np.asarray(inputs["x"], np.float32)
    mem = np.asarray(inputs["memory"], np.float32)
    link = np.asarray(inputs["link_matrix"], np.float32)[:, 0]
    prec = np.asarray(inputs["precedence"], np.float32)[:, 0]
    rw = np.asarray(inputs["read_weights"], np.float32)
    wwin = np.asarray(inputs["write_weights"], np.float32)[:, 0]
    usage = np.asarray(inputs["usage_vector"], np.float32)

    # host-side relayouts so every DMA is contiguous per partition row
    wtf = np.ascontiguousarray(wtf.reshape(9, 128, 6).transpose(1, 0, 2))

    def mpart(a):
        # [M, J] -> [128, M//128, J] (slot chunks on partitions)
        return np.ascontiguousarray(a.reshape(NCH, 128, -1).transpose(1, 0, 2))

    in_maps = []
    for cix in range(NCORES):
        sl = slice(cix * BL, (cix + 1) * BL)
        xta = np.zeros((KIN, BL), np.float32)
        xta[:IN] = x[sl].T
        xta[IN] = 1.0
        xta = np.ascontiguousarray(
            xta.astype(ml_dtypes.bfloat16).reshape(9, 128, BL).transpose(1, 0, 2))
        rws = rw[sl]
        lk = link[sl]
        llt = np.stack([lk, lk.transpose(0, 2, 1)], axis=1) * LSC
        llt = np.ascontiguousarray(
            llt.astype(ml_dtypes.float8_e4m3fn).reshape(BL, 2, NCH, 128, M)
            .transpose(0, 3, 1, 2, 4))
        diag = np.ascontiguousarray(np.diagonal(lk, axis1=1, axis2=2))
        memt = mem[sl].transpose(0, 2, 1).astype(ml_dtypes.bfloat16)
        memt = np.ascontiguousarray(memt.reshape(BL // 2, 2, W, M).transpose(0, 2, 1, 3))
        memn = mem[sl].astype(ml_dtypes.bfloat16)
        memn = np.ascontiguousarray(
            memn.reshape(BL // 2, 2, NCH, 128, W).transpose(0, 3, 1, 2, 4))
        in_maps.append({
            "consts": consts,
            "xta": xta,
            "wta": wta,
            "wtf": wtf,
            "memt": memt,
            "memn": memn,
            "llt": llt,
            "rwt": mpart(rws.transpose(2, 0, 1).reshape(M, BL * R)),
            "prct": mpart(prec[sl].T),
            "wwt": mpart(wwin[sl].T),
            "usgt": mpart(usage[sl].T),
            "diagt": mpart(diag.T),
        })
    return in_maps


def kernel(**inputs):
    nc = build_nc()
    in_maps = make_in_maps(inputs)
    res = run_bass_kernel_spmd(nc, in_maps, list(range(NCORES))).results
    out = np.concatenate([res[c]["out"].reshape(BL, R, W) for c in range(NCORES)], axis=0)
    return out.astype(np.float32)



# revision 12
# speedup vs baseline: 1.0223x; 1.0223x over previous
"""Trainium2 Bass kernel v3 for the DNC memory-step problem (nn_DNC_3882650436261).

Pure data-parallel over batch (128 -> 16 items x 8 cores). Sort-free
allocation, link products via fp8 matmuls against host-prepped L / L^T,
mem_new never materialized. Hardware mapping highlights:

  - "m-part world": the 512-slot dim lives on partitions (4 chunks of 128);
    elementwise/softmax math runs at 128-lane width; slot sums = free-dim
    reduce over chunks + gpsimd partition_all_reduce.
  - allocation mask: u broadcast via PE ones-column matmuls, 64 bf16
    tensor_scalar is_gt compares (DVE), then the masked log-u reduction on
    the PE (cmp block stationary, lnu column streamed) landing m-part
    directly in one PSUM bank.  NB: PSUM accumulation groups must be
    emitted consecutively per output column or start=True data is lost.
  - phase B (mem products) uses memt chunks as the stationary operand so
    results come out slot-partitioned (no staging/transpose); phase F
    (read vectors) uses memn chunks stationary -> w-part results; the
    final combine stays w-part with one PE transpose per item-half.
  - all DMA'd tensors host-relaid so every transfer is contiguous per
    partition row; zero-dependency input loads are emitted before any
    dependent DMA (the sync issue queue is in-order, so a dependent DMA
    blocks every later issue).
  - work is split into item-halves (B/mask/C-head/link in one loop,
    cw/E/F/G in a second) so halves pipeline across engines; the five
    write-side scalars get an early partition broadcast, writers pinned
    to the vector engine to stay ordered with the softplus finalizing ws.

Self-contained: hardcodes all shapes; no file reads.
"""
import os
import numpy as np
import ml_dtypes
from contextlib import ExitStack

import concourse.bass as bass
import concourse.tile as tile
import concourse.bass_isa as bass_isa
from concourse import bacc, mybir
from concourse.bass_utils import run_bass_kernel_spmd

F32 = mybir.dt.float32
BF16 = mybir.dt.bfloat16
F32R = mybir.dt.float32r
FP8 = mybir.dt.float8e4
AF = mybir.ActivationFunctionType
OP = mybir.AluOpType
AX = mybir.AxisListType

B, M, W, R, IN = 128, 512, 128, 4, 1024
NCORES = 8
BL = B // NCORES            # 16 items per core
DELTA = 1e-6
NCH = M // 128              # 4 chunks of the slot dim
KIN = 1152                  # padded contraction dim (1024 + bias row -> 9*128)
DTOT = 919
NS = 34                     # per-item scalar table width
LSC = 512.0                 # fp8 pre-scale for link/rw
LDS = float(2.0 ** -18)     # rescale after fp8 link matmuls

_dims = dict(rk=R * W, rs=R, wk=W, ws=1, ev=W, wv=W, fg=R, ag=1, wg=1, rm=3 * R)
_ofs = {}
_o = 0
for _n, _d in _dims.items():
    _ofs[_n] = _o
    _o += _d
assert _o == DTOT

# scalar-table columns
S_FG, S_RS, S_WS, S_AG, S_WG = 0, 4, 8, 9, 10
S_M0, S_M1, S_M2 = 11, 15, 19
S_BNR, S_BW, S_RKWV, S_C3, S_OMAG = 23, 27, 28, 32, 33


def _emit(nc, tc, ctx, d):
    STOP = int(os.environ.get("KSTOP", "9"))

    P = ctx.enter_context(tc.tile_pool(name="persist", bufs=1))
    ps = ctx.enter_context(tc.tile_pool(name="ps", bufs=5, space=bass.MemorySpace.PSUM))
    psb = ctx.enter_context(tc.tile_pool(name="psb", bufs=2, space=bass.MemorySpace.PSUM))
    psm = ctx.enter_context(tc.tile_pool(name="psm", bufs=1, space=bass.MemorySpace.PSUM))
    stg = ctx.enter_context(tc.tile_pool(name="stg", bufs=2))
    memp = ctx.enter_context(tc.tile_pool(name="memp", bufs=8))
    mtp = ctx.enter_context(tc.tile_pool(name="mtp", bufs=8))
    lp = ctx.enter_context(tc.tile_pool(name="lp", bufs=10))
    scr = ctx.enter_context(tc.tile_pool(name="scr", bufs=8))
    mnp = ctx.enter_context(tc.tile_pool(name="mnp", bufs=8))

    def pst(pr, fr, pool=ps):
        return pool.tile([pr, fr], F32, tag="pst", name="pst")

    _uid = [0]

    def sct(fr=NCH * BL, dt=F32):
        # m-part scratch [128, fr] with a unique persistent allocation
        _uid[0] += 1
        return P.tile([128, fr], dt, tag=f"mp{_uid[0]}", name=f"mp{_uid[0]}")

    gates = P.tile([BL, DTOT], F32, tag="gates")

    def gsl(name, a=0, b=None):
        o = _ofs[name]
        if b is None:
            b = _dims[name]
        return gates[:, o + a:o + b]

    def _bail(lvl, t=None):
        if STOP <= lvl:
            if t is None:
                t = gates[:, 0:R * W]
            nc.sync.dma_start(d["out"][:], t)
            return True
        return False

    # view helpers (m-part free layouts)
    def vci(t):
        return t[:].rearrange("p (c i) -> p c i", i=BL)

    def vcir(t, k=R):
        return t[:].rearrange("p (c i k) -> p c i k", i=BL, k=k)

    # ---------------- constants + inputs resident in SBUF ----------------
    consts = P.tile([128, 129], F32, tag="consts")
    nc.sync.dma_start(consts[:], d["consts"][:])
    I128 = consts[:, 0:128]

    def ptrans(out_psum, in_sb):
        p = in_sb.shape[0]
        nc.tensor.transpose(out_psum, in_sb, I128[0:p, 0:p])

    xta = P.tile([128, 9, BL], BF16, tag="xta")
    nc.sync.dma_start(xta[:], d["xta"][:])
    wtf_sb = P.tile([128, 9, 6], BF16, tag="wtf")
    nc.sync.dma_start(wtf_sb[:], d["wtf"][:])



    # ---------------- phase A: fused linear + gates (item-part) ----------------
    zps = pst(BL, 512)
    zps2 = pst(BL, DTOT - 512)
    zpf = pst(BL, 6, psb)     # priority: fg/ag/wg columns (901:907)
    wp = ctx.enter_context(tc.tile_pool(name="wstream", bufs=9))
    wkts = []
    for k in range(9):
        nc.tensor.matmul(zpf[:], xta[:, k, :], wtf_sb[:, k, :],
                         start=(k == 0), stop=(k == 8))
    for k in range(9):
        wk_t = wp.tile([128, DTOT], BF16, tag="w")
        nc.sync.dma_start(wk_t[:], d["wta"][128 * k:128 * (k + 1), :])
        wkts.append(wk_t)

    RWT = P.tile([128, NCH * BL * R], F32, tag="RWT")    # rw^T [p,(c,i,r)]
    nc.sync.dma_start(RWT[:], d["rwt"][:])
    PRC = P.tile([128, NCH * BL], F32, tag="PRC")        # precedence^T
    nc.sync.dma_start(PRC[:], d["prct"][:])
    WWI = P.tile([128, NCH * BL], F32, tag="WWI")        # write_weights in ^T
    nc.sync.dma_start(WWI[:], d["wwt"][:])
    USG = P.tile([128, NCH * BL], F32, tag="USG")        # usage in ^T
    nc.sync.dma_start(USG[:], d["usgt"][:])
    DGT = P.tile([128, NCH * BL], F32, tag="DGT")        # diag(L) ^T
    nc.sync.dma_start(DGT[:], d["diagt"][:])

    # memory loads up front: zero-dependency DMAs must precede dependent ones
    # on the in-order sync issue queue
    mem2s, mn2s = [], []
    for j in range(BL // 2):
        mem2 = memp.tile([128, 2, M], BF16, tag="memt")
        nc.sync.dma_start(mem2[:], d["memt"][j])
        mem2s.append(mem2)
    for j in range(BL // 2):
        mn2 = mnp.tile([128, 2, NCH, W], BF16, tag="memn")
        nc.sync.dma_start(mn2[:], d["memn"][j])
        mn2s.append(mn2)

    def zsl(a, b):
        if b <= 512:
            return zps[:, a:b]
        assert a >= 512
        return zps2[:, a - 512:b - 512]

    nc.scalar.activation(gates[:, 901:907], zpf[:], AF.Sigmoid)           # fg,ag,wg (priority)
    fgrow = P.tile([1, BL * R], F32, tag="fgrow")
    nc.scalar.dma_start(fgrow[:], gsl("fg"))
    FGB = P.tile([128, BL * R], F32, tag="FGB")
    nc.gpsimd.partition_broadcast(FGB[:], fgrow[:])
    # ---------------- phase A2: usage / u / log u (m-part) ----------------
    fgb = FGB[:].rearrange("p (o i r) -> p o i r", o=1, r=R).broadcast_to([128, NCH, BL, R])
    psi4 = sct(NCH * BL * R)
    nc.vector.tensor_tensor(vcir(psi4), vcir(RWT), fgb, op=OP.mult)
    nc.any.tensor_scalar(psi4[:], psi4[:], 1.0, None, op0=OP.subtract)  # fg*rw - 1
    psi = sct()
    p4 = vcir(psi4)
    nc.vector.tensor_tensor(vci(psi), p4[:, :, :, 0], p4[:, :, :, 1], op=OP.mult)
    nc.vector.tensor_tensor(p4[:, :, :, 2], p4[:, :, :, 2], p4[:, :, :, 3], op=OP.mult)
    nc.vector.tensor_tensor(vci(psi), vci(psi), p4[:, :, :, 2], op=OP.mult)

    u_sb = P.tile([128, NCH * BL], F32, tag="u_sb")
    nc.vector.tensor_tensor(u_sb[:], USG[:], WWI[:], op=OP.mult)
    nc.vector.tensor_tensor(u_sb[:], USG[:], u_sb[:], op=OP.subtract)
    nc.vector.tensor_tensor(u_sb[:], u_sb[:], WWI[:], op=OP.add)
    nc.vector.tensor_tensor(u_sb[:], u_sb[:], psi[:], op=OP.mult)
    nc.any.tensor_scalar(u_sb[:], u_sb[:], 1.0 - DELTA, DELTA, op0=OP.mult, op1=OP.add)

    # u_ip (item-part copy of u) -> u_flat single row for mask broadcasts
    uips = pst(BL, M, psb)
    for c in range(NCH):
        ptrans(uips[:, 128 * c:128 * (c + 1)], vci(u_sb)[:, c, :])
    u_ip = P.tile([BL, M], F32, tag="u_ip")
    nc.any.tensor_copy(u_ip[:], uips[:])
    ub_ip = P.tile([BL, M], BF16, tag="ub_ip")
    nc.any.tensor_copy(ub_ip[:], uips[:])
    u_bf = P.tile([128, NCH * BL], BF16, tag="u_bf")
    nc.vector.tensor_copy(u_bf[:], u_sb[:])
    # one row tile per 4-item quarter so each UREPA broadcast matmul depends
    # on only its own two DMAs instead of all eight (whole-tile dependency)
    u_flatq = []
    for q in range(4):
        ufq = P.tile([1, 4 * M], BF16, tag=f"u_flat{q}", name=f"u_flat{q}")
        for j in range(2):
            nc.sync.dma_start(ufq[0:1, 1024 * j:1024 * (j + 1)],
                              ub_ip[4 * q + 2 * j:4 * q + 2 * j + 2, :])
        u_flatq.append(ufq)

    if _bail(2, u_ip[:, 0:R * W]):
        return

    for k in range(9):
        nc.tensor.matmul(zps[:], xta[:, k, :], wkts[k][:, 0:512],
                         start=(k == 0), stop=(k == 8))
        nc.tensor.matmul(zps2[:], xta[:, k, :], wkts[k][:, 512:DTOT],
                         start=(k == 0), stop=(k == 8))

    nc.scalar.activation(gates[:, 645:773], zsl(645, 773), AF.Sigmoid)           # ev
    nc.scalar.activation(gates[:, 0:512], zsl(0, 512), AF.Tanh)                  # rk
    nc.scalar.activation(gates[:, 516:644], zsl(516, 644), AF.Tanh)              # wk
    nc.scalar.activation(gates[:, 773:901], zsl(773, 901), AF.Tanh)              # wv
    nc.vector.tensor_copy(gates[:, 907:919], zsl(907, 919))                      # rm logits
    # softplus(z) = -ln(sigmoid(-z)): sigmoid rides the ev table load, one Ln load
    spts = {}
    for (a, b) in [(512, 516), (644, 645)]:
        spts[a] = scr.tile([BL, b - a], F32, tag="sp", name="spt")
        nc.scalar.activation(spts[a][:], zsl(a, b), AF.Sigmoid, scale=-1.0)
    for (a, b) in [(512, 516), (644, 645)]:
        nc.scalar.activation(spts[a][:], spts[a][:], AF.Ln)
    for (a, b) in [(512, 516), (644, 645)]:
        nc.vector.tensor_scalar(gates[:, a:b], spts[a][:], -1.0, None, op0=OP.mult)



    # read-mode softmax over groups of 3
    rmz = gates[:, 907:919].rearrange("i (r k) -> i r k", k=3)
    negmax3 = P.tile([BL, R], F32, tag="negmax3")
    nc.vector.tensor_reduce(negmax3[:], rmz, axis=AX.X, op=OP.max, negate=True)
    rme = P.tile([BL, 3 * R], F32, tag="rme")
    nc.vector.tensor_tensor(rme[:].rearrange("i (r k) -> i r k", k=3), rmz,
                            negmax3[:].rearrange("i (r o) -> i r o", o=1).broadcast_to([BL, R, 3]),
                            op=OP.add)
    nc.scalar.activation(rme[:], rme[:], AF.Exp)
    rmsum = P.tile([BL, R], F32, tag="rmsum")
    nc.vector.tensor_reduce(rmsum[:], rme[:].rearrange("i (r k) -> i r k", k=3), axis=AX.X, op=OP.add)
    nc.vector.reciprocal(rmsum[:], rmsum[:])
    rm = P.tile([BL, 3 * R], F32, tag="rm")
    nc.vector.tensor_tensor(rm[:].rearrange("i (r k) -> i r k", k=3),
                            rme[:].rearrange("i (r k) -> i r k", k=3),
                            rmsum[:].rearrange("i (r o) -> i r o", o=1).broadcast_to([BL, R, 3]),
                            op=OP.mult)

    # per-item key-norm scalars (item-part)
    scw = P.tile([BL, W], F32, tag="scw")
    bw128 = P.tile([BL, 1], F32, tag="bw128")
    nc.scalar.activation(scw[:], gsl("wk"), AF.Square, accum_out=bw128[:])
    nc.scalar.activation(bw128[:], bw128[:], AF.Sqrt)
    nc.any.tensor_scalar(bw128[:], bw128[:], float(W), float(W) * DELTA, op0=OP.mult, op1=OP.add)

    # early broadcast of the 5 write-side scalars the C-head needs (the full
    # SCB table waits on read-side norms which are only needed by phase E)
    NSW = 5
    W_WS, W_AG, W_WG, W_BW, W_OMAG = 0, 1, 2, 3, 4
    # all writers pinned to vector so they are in-order with the softplus add
    # that finalizes gates["ws"] (cross-engine timing there proved racy)
    SCALW = P.tile([BL, NSW], F32, tag="SCALW")
    nc.vector.tensor_copy(SCALW[:, W_WS:W_WS + 1], gsl("ws"))
    nc.vector.tensor_copy(SCALW[:, W_AG:W_AG + 1], gsl("ag"))
    nc.vector.tensor_copy(SCALW[:, W_WG:W_WG + 1], gsl("wg"))
    nc.vector.tensor_copy(SCALW[:, W_BW:W_BW + 1], bw128[:])
    nc.vector.tensor_scalar(SCALW[:, W_OMAG:W_OMAG + 1], gsl("ag"), -1.0, 1.0,
                            op0=OP.mult, op1=OP.add)
    scalwrow = P.tile([1, BL * NSW], F32, tag="scalwrow")
    nc.sync.dma_start(scalwrow[:], SCALW[:])
    SCW = P.tile([128, BL * NSW], F32, tag="SCW")
    nc.gpsimd.partition_broadcast(SCW[:], scalwrow[:])

    def scw_c(s):
        return SCW[:].rearrange("p (o i s) -> p o i s", o=1, s=NSW)[:, :, :, s] \
            .broadcast_to([128, NCH, BL])

    bnr = P.tile([BL, R], F32, tag="bnr")
    rkwv = P.tile([BL, R], F32, tag="rkwv")
    for r in range(R):
        nc.scalar.activation(scw[:], gsl("rk", r * W, (r + 1) * W), AF.Square, accum_out=bnr[:, r:r + 1])
        nc.vector.tensor_tensor(scw[:], gsl("rk", r * W, (r + 1) * W), gsl("wv"), op=OP.mult)
        nc.vector.tensor_scalar(scw[:], scw[:], 1.0, None, op0=OP.mult, op1=OP.add,
                                accum_out=rkwv[:, r:r + 1])
    nc.scalar.activation(bnr[:], bnr[:], AF.Sqrt)
    nc.any.tensor_scalar(bnr[:], bnr[:], float(W), float(W) * DELTA, op0=OP.mult, op1=OP.add)
    c3 = P.tile([BL, 1], F32, tag="c3")
    nc.scalar.activation(scw[:], gsl("wv"), AF.Square, accum_out=c3[:])

    # KCM/NRM lhsT tables (w on partitions), f32 then cast to bf16
    KCMf = P.tile([128, BL * 11], F32, tag="KCMf")
    NRMf = P.tile([128, BL * 3], F32, tag="NRMf")
    EVT = P.tile([128, BL], F32, tag="EVT")

    def kcm_col(j):
        return KCMf[:].rearrange("p (i k) -> p i k", k=11)[:, :, j]

    gtp = pst(128, BL, psb)
    ptrans(gtp[:], gsl("wk"))
    nc.any.tensor_copy(kcm_col(0), gtp[:])
    gtp = pst(128, BL, psb)
    ptrans(gtp[:], gsl("ev"))
    nc.any.tensor_copy(EVT[:], gtp[:])
    gtp = pst(128, BL, psb)
    ptrans(gtp[:], gsl("wv"))
    nc.any.tensor_copy(kcm_col(9), gtp[:])
    nc.vector.tensor_tensor(kcm_col(10), kcm_col(9), EVT[:], op=OP.mult)  # ev*wv
    for r in range(R):
        gtp = pst(128, BL, psb)
        ptrans(gtp[:], gsl("rk", r * W, (r + 1) * W))
        nc.any.tensor_copy(kcm_col(1 + r), gtp[:])
        nc.vector.tensor_tensor(kcm_col(5 + r), kcm_col(1 + r), EVT[:], op=OP.mult)
    nrm3 = NRMf[:].rearrange("p (i k) -> p i k", k=3)
    nc.any.memset(nrm3[:, :, 0], 1.0)
    nc.any.tensor_copy(nrm3[:, :, 1], EVT[:])
    nc.scalar.activation(nrm3[:, :, 2], EVT[:], AF.Square)
    KCMb = P.tile([128, BL * 11], BF16, tag="KCMb")
    nc.any.tensor_copy(KCMb[:], KCMf[:])
    NRMb = P.tile([128, BL * 3], BF16, tag="NRMb")
    nc.any.tensor_copy(NRMb[:], NRMf[:])

    # ---------------- scalar table -> partition-broadcast SCB ----------------
    SCAL = P.tile([BL, NS], F32, tag="SCAL")
    nc.any.tensor_copy(SCAL[:, S_FG:S_FG + R], gsl("fg"))
    nc.any.tensor_copy(SCAL[:, S_RS:S_RS + R], gsl("rs"))
    nc.any.tensor_copy(SCAL[:, S_WS:S_WS + 1], gsl("ws"))
    nc.any.tensor_copy(SCAL[:, S_AG:S_AG + 1], gsl("ag"))
    nc.any.tensor_copy(SCAL[:, S_WG:S_WG + 1], gsl("wg"))
    for k in range(3):
        nc.any.tensor_copy(SCAL[:, S_M0 + R * k:S_M0 + R * (k + 1)],
                           rm[:].rearrange("i (r k) -> i r k", k=3)[:, :, k])
    nc.any.tensor_copy(SCAL[:, S_BNR:S_BNR + R], bnr[:])
    nc.any.tensor_copy(SCAL[:, S_BW:S_BW + 1], bw128[:])
    nc.any.tensor_copy(SCAL[:, S_RKWV:S_RKWV + R], rkwv[:])
    nc.any.tensor_copy(SCAL[:, S_C3:S_C3 + 1], c3[:])
    nc.any.tensor_scalar(SCAL[:, S_OMAG:S_OMAG + 1], gsl("ag"), -1.0, 1.0,
                         op0=OP.mult, op1=OP.add)
    scalrow = P.tile([1, BL * NS], F32, tag="scalrow")
    nc.scalar.dma_start(scalrow[:], SCAL[:])
    SCB = P.tile([128, BL * NS], F32, tag="SCB")
    nc.gpsimd.partition_broadcast(SCB[:], scalrow[:])

    def scb(s, w=1):
        # [128, BL, w] view of scalar cols s..s+w
        return SCB[:].rearrange("p (i s) -> p i s", s=NS)[:, :, s:s + w]

    def scb_c(s):
        # broadcast over chunks -> [128, NCH, BL]
        return SCB[:].rearrange("p (o i s) -> p o i s", o=1, s=NS)[:, :, :, s] \
            .broadcast_to([128, NCH, BL])

    def scb_cr(s):
        # per-(i,r) scalars broadcast over chunks -> [128, NCH, BL, R]
        return SCB[:].rearrange("p (o i s) -> p o i s", o=1, s=NS)[:, :, :, s:s + R] \
            .broadcast_to([128, NCH, BL, R])

    def bc_r(t):
        # [128, NCH*BL] -> [128, NCH, BL, R] broadcast over r
        return t[:].rearrange("p (c i o) -> p c i o", i=BL, o=1).broadcast_to([128, NCH, BL, R])

    # ---------------- allocation mask v2 (ts compare + PE reduce) ----------------
    # Emitted AFTER the SCAL/SCB section so the 64 is_gt ops don't starve the
    # vector-engine ops feeding the scalar-table broadcast (C-head dependency).
    # S_acc[p,(c,i)] = sum_j [u_i[j] < u_i[slot(c,p)]] * ln u_i[j], computed as:
    #   cmp_t[j, p'] = (u[p'] > u[j])  (tensor_scalar is_gt, bf16)
    #   S col (cp,i) = sum_{cj} cmp_t_block^T @ lnu_col   (PE, m-part direct)
    # broadcast u_flat across partitions via PE ones-column matmuls (PE is idle
    # here; the serial gpsimd broadcasts were 3.2us each on the critical path)
    ones_row = P.tile([1, 128], BF16, tag="ones_row")
    nc.vector.memset(ones_row[:], 1.0)
    NQ = 4
    IQ = BL // NQ
    urepq = []
    for q in range(NQ):
        uq = P.tile([128, IQ * M], BF16, tag=f"UREPA{q}", name=f"UREPA{q}")
        for s in range(IQ):
            i = IQ * q + s
            ubx = pst(128, M, psb)
            nc.tensor.matmul(ubx[:], ones_row[:],
                             u_flatq[i // 4][0:1, M * (i % 4):M * (i % 4 + 1)],
                             start=True, stop=True)
            if i % 2 == 0:
                nc.scalar.activation(uq[:, M * s:M * (s + 1)], ubx[:], AF.Copy)
            else:
                nc.vector.tensor_copy(uq[:, M * s:M * (s + 1)], ubx[:])
        urepq.append(uq)
    LNU = P.tile([128, NCH * BL], BF16, tag="LNU")
    nc.scalar.activation(LNU[:], u_sb[:], AF.Ln)
    # u rounded to bf16 then held in f32, so the is_gt scalar sees the same
    # rounding as the bf16 in0 (a slot must not compare unequal to itself)
    u_bf32 = P.tile([128, NCH * BL], F32, tag="u_bf32")
    nc.vector.tensor_copy(u_bf32[:], u_bf[:])
    S_PS = psm.tile([128, NCH * BL], F32, tag="sps", name="sps")
    sps = S_PS[:].rearrange("p (c i) -> p c i", i=BL)

    def emit_mask_items(items):
        for i in items:
            cts = []
            for cj in range(NCH):
                cmp_t = scr.tile([128, M], BF16, tag="mscr", name="mscr")
                nc.vector.tensor_scalar(cmp_t[:], urepq[i // IQ][:, M * (i % IQ):M * (i % IQ + 1)],
                                        vci(u_bf32)[:, cj, i:i + 1], None, op0=OP.is_gt)
                cts.append(cmp_t)
            for cp in range(NCH):
                for cj in range(NCH):
                    nc.tensor.matmul(sps[:, cp, i:i + 1], cts[cj][:, 128 * cp:128 * (cp + 1)],
                                     LNU[:, cj * BL + i:cj * BL + i + 1],
                                     start=(cj == 0), stop=(cj == NCH - 1))

    if _bail(1):
        return

    # ---------------- phase B: memory products (per item, m-part out) ----------------
    # out[m-chunk, col] = sum_w memt[w, m]*KCM[w, col]  (and mt2 for norms):
    # the matmul produces slot-partitioned results directly; one small strided
    # PSUM->SBUF copy per item replaces the old stage+dma_transpose pipeline.
    NB = 14
    BCOLL = P.tile([128, NCH * BL * NB], BF16, tag="BCOLL")
    mt2s = []
    for i in range(BL):
        mt2 = mtp.tile([128, M], BF16, tag="mt2")
        nc.scalar.activation(mt2[:], mem2s[i // 2][:, i % 2, :], AF.Square)
        mt2s.append(mt2)
    def emit_B_items(items):
        for i in items:
            mti = mem2s[i // 2][:, i % 2, :]
            bp = pst(128, NCH * NB)
            for c in range(NCH):
                nc.tensor.matmul(bp[:, NB * c:NB * c + 11], mti[:, 128 * c:128 * (c + 1)],
                                 KCMb[:, 11 * i:11 * (i + 1)], start=True, stop=True)
                nc.tensor.matmul(bp[:, NB * c + 11:NB * c + 14],
                                 mt2s[i][:, 128 * c:128 * (c + 1)],
                                 NRMb[:, 3 * i:3 * (i + 1)], start=True, stop=True)
            dst = BCOLL[:].rearrange("p (c i k) -> p i c k", i=BL, k=NB)[:, i, :, :]
            nc.scalar.activation(dst, bp[:].rearrange("p (c k) -> p c k", k=NB), AF.Copy)

    # ---------------- phase C head: alloc / wcw / ww / fp8 lhsT ----------------
    bcf = vcir(BCOLL, NB)

    def bcol(j):
        return bcf[:, :, :, j]

    WCN, T1, T2, S0, S1, S2 = bcol(0), bcol(9), bcol(10), bcol(11), bcol(12), bcol(13)

    EXS, onemu, alloc = sct(), sct(), sct()
    AO, wden, wz, wcw = sct(), sct(), sct(), sct()
    wzs = P.tile([128, BL], F32, tag="wzs")
    WZS = P.tile([128, BL], F32, tag="WZS")
    ww = P.tile([128, NCH * BL], F32, tag="ww")
    RWC8 = P.tile([128, NCH * BL * 8], FP8, tag="RWC8")
    rwc = vcir(RWC8, 8)
    DCOLL = P.tile([128, NCH * BL * 64], BF16, tag="DCOLL")

    for h in range(2):
        sl = slice(8 * h, 8 * h + 8)
        # --- phase-B half first so its scalar copies precede this half's
        #     C-head ops in the in-order scalar queue ---
        emit_B_items(range(8 * h, 8 * h + 8))
        # --- allocation-mask half (is_gt + PE reduce for these items) ---
        emit_mask_items(range(8 * h, 8 * h + 8))
        # --- C-head for this half ---
        nc.scalar.activation(vci(EXS)[:, :, sl], sps[:, :, sl], AF.Exp)
        nc.vector.tensor_scalar(vci(onemu)[:, :, sl], vci(u_sb)[:, :, sl], -1.0, 1.0,
                                op0=OP.mult, op1=OP.add)
        nc.vector.tensor_tensor(vci(alloc)[:, :, sl], vci(onemu)[:, :, sl],
                                vci(EXS)[:, :, sl], op=OP.mult)
        nc.scalar.activation(vci(AO)[:, :, sl], S0[:, :, sl], AF.Sqrt)
        nc.vector.tensor_scalar(vci(AO)[:, :, sl], vci(AO)[:, :, sl], 1.0, DELTA,
                                op0=OP.mult, op1=OP.add)
        nc.vector.tensor_tensor(vci(wden)[:, :, sl], vci(AO)[:, :, sl],
                                scw_c(W_BW)[:, :, sl], op=OP.mult)
        nc.vector.tensor_scalar(vci(wden)[:, :, sl], vci(wden)[:, :, sl], 1.0, DELTA,
                                op0=OP.mult, op1=OP.add)
        nc.vector.reciprocal(vci(wden)[:, :, sl], vci(wden)[:, :, sl])
        nc.vector.tensor_tensor(vci(wz)[:, :, sl], WCN[:, :, sl], vci(wden)[:, :, sl],
                                op=OP.mult)
        nc.vector.tensor_tensor(vci(wz)[:, :, sl], vci(wz)[:, :, sl],
                                scw_c(W_WS)[:, :, sl], op=OP.mult)
        nc.scalar.activation(vci(wz)[:, :, sl], vci(wz)[:, :, sl], AF.Exp)
        nc.vector.tensor_reduce(wzs[:, sl], wz[:].rearrange("p (c i) -> p i c", i=BL)[:, sl, :],
                                axis=AX.X, op=OP.add)
        nc.gpsimd.partition_all_reduce(WZS[:, sl], wzs[:, sl], channels=128,
                                       reduce_op=bass_isa.ReduceOp.add)
        nc.vector.reciprocal(WZS[:, sl], WZS[:, sl])
        nc.vector.tensor_tensor(vci(wcw)[:, :, sl], vci(wz)[:, :, sl],
                                WZS[:].rearrange("p (o i) -> p o i", o=1)[:, :, sl]
                                .broadcast_to([128, NCH, 8]), op=OP.mult)
        nc.vector.tensor_tensor(vci(alloc)[:, :, sl], vci(alloc)[:, :, sl],
                                scw_c(W_AG)[:, :, sl], op=OP.mult)
        nc.vector.tensor_tensor(vci(ww)[:, :, sl], vci(wcw)[:, :, sl],
                                scw_c(W_OMAG)[:, :, sl], op=OP.mult)
        nc.vector.tensor_tensor(vci(ww)[:, :, sl], vci(ww)[:, :, sl],
                                vci(alloc)[:, :, sl], op=OP.add)
        nc.vector.tensor_tensor(vci(ww)[:, :, sl], vci(ww)[:, :, sl],
                                scw_c(W_WG)[:, :, sl], op=OP.mult)
        for c in range(NCH):
            nc.vector.tensor_scalar(rwc[:, c, sl, 0:4], vcir(RWT)[:, c, sl, :], LSC, None,
                                    op0=OP.mult)
            nc.vector.scalar_tensor_tensor(rwc[:, c, sl, 4:8], vcir(RWT)[:, c, sl, :], LSC,
                                           bc_r(ww)[:, c, sl, :], op0=OP.mult, op1=OP.mult)
        # --- D for this half ---
        stg2d = None
        for i in range(8 * h, 8 * h + 8):
            ll = lp.tile([128, 2, NCH, M], FP8, tag="ll")
            nc.sync.dma_start(ll[:], d["llt"][i])
            if i % 2 == 0:
                stg2d = stg.tile([128, M], BF16, tag="stg2", name="stgD")
            bps = pst(8, M)
            fps = pst(8, M)
            for cp in range(NCH // 2):
                lhs2 = RWC8[:].rearrange("p (c i k) -> p c i k", i=BL, k=8)[:, 2 * cp:2 * cp + 2, i, :]
                nc.tensor.matmul(bps[:], lhs2, ll[:, 0, 2 * cp:2 * cp + 2, :],
                                 start=(cp == 0), stop=(cp == 1),
                                 perf_mode=mybir.MatmulPerfMode.DoubleRow)
                nc.tensor.matmul(fps[:], lhs2, ll[:, 1, 2 * cp:2 * cp + 2, :],
                                 start=(cp == 0), stop=(cp == 1),
                                 perf_mode=mybir.MatmulPerfMode.DoubleRow)
            o = 64 * (i % 2)
            nc.scalar.activation(stg2d[o:o + 8, :], bps[:], AF.Copy, scale=LDS)
            nc.vector.tensor_scalar(stg2d[o + 32:o + 40, :], fps[:], LDS, None, op0=OP.mult)
            if i % 2 == 1:
                dst = DCOLL[:].rearrange("p (c i k) -> p c (i k)", i=BL, k=64)[:, :, 64 * (i - 1):64 * (i + 1)]
                nc.sync.dma_start_transpose(dst, stg2d[:])

    if _bail(6, u_ip[:, 0:R * W]):
        return





    # ---------------- phases E+F per item-half (pipelined with D) ----------------
    BH = 8
    dcv = vcir(DCOLL, 64)
    # one output tile per item-half so each half's out-DMA depends only on
    # its own G copy (DMA reads appear to track whole tiles)
    out_sbh = [P.tile([BL * R // 2, W], F32, tag=f"out_sb{h}", name=f"out_sb{h}")
               for h in range(2)]
    RVL = P.tile([128, NCH * BL * 8], BF16, tag="RVL")
    rvv = vcir(RVL, 8)


    def sctH(fr=NCH * BH, dt=F32):
        _uid[0] += 1
        return P.tile([128, fr], dt, tag=f"mh{_uid[0]}", name=f"mh{_uid[0]}")

    def vciH(t):
        return t[:].rearrange("p (c i) -> p c i", i=BH)

    def vcirH(t, k=R):
        return t[:].rearrange("p (c i k) -> p c i k", i=BH, k=k)

    def mk_bcH(sl):
        def bcH(x):
            # slice of full m-part [128, NCH*BL] -> bcast [128, NCH, BH, R]
            return x[:].rearrange("p (c i o) -> p c i o", i=BL, o=1)[:, :, sl, :] \
                .broadcast_to([128, NCH, BH, R])
        return bcH

    def bcHt(t):
        # per-half tile [128, NCH*BH] -> bcast over r
        return t[:].rearrange("p (c i o) -> p c i o", i=BH, o=1) \
            .broadcast_to([128, NCH, BH, R])

    cwm2s = []
    for h in range(2):
        sl = slice(BH * h, BH * (h + 1))
        bcH = mk_bcH(sl)
        wwH = vci(ww)[:, :, sl]
        S0h, S1h, S2h = S0[:, :, sl], S1[:, :, sl], S2[:, :, sl]
        T1h, T2h = T1[:, :, sl], T2[:, :, sl]

        # new-memory norms AN
        ww2 = sctH()
        nc.vector.tensor_tensor(vciH(ww2), wwH, wwH, op=OP.mult)
        q1 = sctH()
        nc.vector.tensor_tensor(vciH(q1), S1h, T1h, op=OP.subtract)
        nc.vector.scalar_tensor_tensor(vciH(q1), wwH, -2.0, vciH(q1), op0=OP.mult, op1=OP.mult)
        q2 = sctH()
        nc.vector.scalar_tensor_tensor(vciH(q2), T2h, -2.0, S2h, op0=OP.mult, op1=OP.add)
        nc.vector.tensor_tensor(vciH(q2), vciH(q2), scb_c(S_C3)[:, :, sl], op=OP.add)
        nc.vector.tensor_tensor(q2[:], q2[:], ww2[:], op=OP.mult)
        AN = sctH()
        nc.vector.tensor_tensor(vciH(AN), S0h, vciH(q1), op=OP.add)
        nc.vector.tensor_tensor(AN[:], AN[:], q2[:], op=OP.add)
        nc.scalar.activation(AN[:], AN[:], AF.Sqrt)
        nc.any.tensor_scalar(AN[:], AN[:], 1.0, DELTA, op0=OP.mult, op1=OP.add)

        # read content weights cw (scaled by mode2 / csum)
        cnum = sctH(NCH * BH * R)
        cn = vcirH(cnum)
        cwA = bcf[:, :, sl, 1:5]
        cwB = bcf[:, :, sl, 5:9]
        nc.vector.tensor_tensor(cn, cwB, bcH(ww), op=OP.mult)
        nc.vector.tensor_tensor(cn, cwA, cn, op=OP.subtract)
        ct = sctH(NCH * BH * R)
        nc.vector.tensor_tensor(vcirH(ct), bcH(ww), scb_cr(S_RKWV)[:, :, sl, :], op=OP.mult)
        nc.vector.tensor_tensor(cnum[:], cnum[:], ct[:], op=OP.add)
        cden = sctH(NCH * BH * R)
        nc.vector.tensor_tensor(vcirH(cden), bcHt(AN), scb_cr(S_BNR)[:, :, sl, :], op=OP.mult)
        nc.any.tensor_scalar(cden[:], cden[:], 1.0, DELTA, op0=OP.mult, op1=OP.add)
        nc.vector.reciprocal(cden[:], cden[:])
        nc.vector.tensor_tensor(cnum[:], cnum[:], cden[:], op=OP.mult)
        nc.vector.tensor_tensor(cn, cn, scb_cr(S_RS)[:, :, sl, :], op=OP.mult)
        nc.scalar.activation(cnum[:], cnum[:], AF.Exp)
        csum = sctH(BH * R)
        nc.vector.tensor_reduce(csum[:], cnum[:].rearrange("p (c j) -> p j c", j=BH * R),
                                axis=AX.X, op=OP.add)
        CSR = sctH(BH * R)
        nc.gpsimd.partition_all_reduce(CSR[:], csum[:], channels=128,
                                       reduce_op=bass_isa.ReduceOp.add)
        nc.vector.reciprocal(CSR[:], CSR[:])
        nc.vector.tensor_tensor(CSR[:].rearrange("p (i r) -> p i r", r=R),
                                CSR[:].rearrange("p (i r) -> p i r", r=R),
                                scb(S_M2, R)[:, sl, :], op=OP.mult)
        cwm2 = sctH(NCH * BH * R)   # mode2 * cw
        nc.vector.tensor_tensor(vcirH(cwm2), cn,
                                CSR[:].rearrange("p (o i r) -> p o i r", o=1, r=R)
                                .broadcast_to([128, NCH, BH, R]), op=OP.mult)
        cwm2s.append(cwm2)

    for h in range(2):
        sl = slice(BH * h, BH * (h + 1))
        bcH = mk_bcH(sl)
        wwH = vci(ww)[:, :, sl]
        cwm2 = cwm2s[h]

        # ---- phase E: assemble fwd/bwd/rw_new (m-part) ----
        P1, P2 = dcv[:, :, sl, 0:4], dcv[:, :, sl, 4:8]
        F1, F2 = dcv[:, :, sl, 32:36], dcv[:, :, sl, 36:40]
        rwtH = vcir(RWT)[:, :, sl, :]
        prcbH = PRC[:].rearrange("p (c i o) -> p c i o", i=BL, o=1)[:, :, sl, :] \
            .broadcast_to([128, NCH, BH, R])

        # cpr = prec . rw_r ; dwr = rw_r . ww   (per item, read head)
        scr4 = sctH(NCH * BH * R)
        nc.vector.tensor_tensor(vcirH(scr4), rwtH, prcbH, op=OP.mult)
        CDW = sctH(2 * BH * R)
        nc.vector.tensor_reduce(CDW[:, 0:BH * R],
                                scr4[:].rearrange("p (c j) -> p j c", j=BH * R),
                                axis=AX.X, op=OP.add)
        scr4b = sctH(NCH * BH * R)
        nc.vector.tensor_tensor(vcirH(scr4b), rwtH, bcH(ww), op=OP.mult)
        nc.vector.tensor_reduce(CDW[:, BH * R:2 * BH * R],
                                scr4b[:].rearrange("p (c j) -> p j c", j=BH * R),
                                axis=AX.X, op=OP.add)
        CDWr = sctH(2 * BH * R)
        nc.gpsimd.partition_all_reduce(CDWr[:], CDW[:], channels=128,
                                       reduce_op=bass_isa.ReduceOp.add)

        def cdw_b(off):
            return CDWr[:, off:off + BH * R].rearrange("p (o i r) -> p o i r", o=1, r=R) \
                .broadcast_to([128, NCH, BH, R])

        # dv = (1-2ww)*diag + ww*prec ; DR = rw * dv
        dv = sctH()
        nc.vector.tensor_scalar(vciH(dv), wwH, -2.0, 1.0, op0=OP.mult, op1=OP.add)
        nc.vector.tensor_tensor(vciH(dv), vciH(dv), vci(DGT)[:, :, sl], op=OP.mult)
        t2m = sctH()
        nc.vector.tensor_tensor(vciH(t2m), wwH, vci(PRC)[:, :, sl], op=OP.mult)
        nc.vector.tensor_tensor(dv[:], dv[:], t2m[:], op=OP.add)
        DR = sctH(NCH * BH * R)
        nc.vector.tensor_tensor(vcirH(DR), rwtH, bcHt(dv), op=OP.mult)

        onemw = sctH()
        nc.vector.tensor_scalar(vciH(onemw), wwH, -1.0, 1.0, op0=OP.mult, op1=OP.add)

        # fwd = F1*(1-ww) - F2 + ww (x) cpr - DR   (then scaled by mode1)
        fwd = sctH(NCH * BH * R)
        fv = vcirH(fwd)
        nc.vector.tensor_tensor(fv, F1, bcHt(onemw), op=OP.mult)
        nc.vector.tensor_tensor(fv, fv, F2, op=OP.subtract)
        ftt = sctH(NCH * BH * R)
        nc.vector.tensor_tensor(vcirH(ftt), bcH(ww), cdw_b(0), op=OP.mult)
        nc.vector.tensor_tensor(fwd[:], fwd[:], ftt[:], op=OP.add)
        nc.vector.tensor_tensor(fwd[:], fwd[:], DR[:], op=OP.subtract)

        # bwd = P1*(1-ww) - P2 + prec (x) dwr - DR  (then scaled by mode0)
        bwd = sctH(NCH * BH * R)
        bv = vcirH(bwd)
        nc.vector.tensor_tensor(bv, P1, bcHt(onemw), op=OP.mult)
        nc.vector.tensor_tensor(bv, bv, P2, op=OP.subtract)
        nc.vector.tensor_tensor(vcirH(ftt), prcbH, cdw_b(BH * R), op=OP.mult)
        nc.vector.tensor_tensor(bwd[:], bwd[:], ftt[:], op=OP.add)
        nc.vector.tensor_tensor(bwd[:], bwd[:], DR[:], op=OP.subtract)

        rwnew = sctH(NCH * BH * R)
        nc.vector.tensor_tensor(bv, bv, scb_cr(S_M0)[:, :, sl, :], op=OP.mult)
        nc.vector.tensor_tensor(fv, fv, scb_cr(S_M1)[:, :, sl, :], op=OP.mult)
        nc.vector.tensor_tensor(rwnew[:], bwd[:], fwd[:], op=OP.add)
        nc.vector.tensor_tensor(rwnew[:], rwnew[:], cwm2[:], op=OP.add)

        # sc = rwnew . ww
        nc.vector.tensor_tensor(vcirH(scr4), vcirH(rwnew), bcH(ww), op=OP.mult)
        SC1 = sctH(BH * R)
        nc.vector.tensor_reduce(SC1[:], scr4[:].rearrange("p (c j) -> p j c", j=BH * R),
                                axis=AX.X, op=OP.add)
        SCR_ = sctH(BH * R)
        nc.gpsimd.partition_all_reduce(SCR_[:], SC1[:], channels=128,
                                       reduce_op=bass_isa.ReduceOp.add)

        # bf16 lhsT for read vectors: [rwnew | rwnew*ww]
        nc.any.tensor_copy(rvv[:, :, sl, 0:4], vcirH(rwnew))
        nc.vector.tensor_tensor(rvv[:, :, sl, 4:8], vcirH(rwnew), bcH(ww), op=OP.mult)

        # ---- phase F: read vectors, w-part out (lhsT = memn chunk) ----
        # trp[w, k] = sum_m mem[m, w] * rvl[m, k]; k = [rw_new heads | rw_new*ww heads]
        TRH = sctH(BH * 8)
        for i in range(BH * h, BH * (h + 1)):
            trp = pst(128, 8)
            for c in range(NCH):
                nc.tensor.matmul(trp[:], mn2s[i // 2][:, i % 2, c, :], rvv[:, c, i, :],
                                 start=(c == 0), stop=(c == NCH - 1))
            nc.vector.tensor_copy(TRH[:, 8 * (i - BH * h):8 * (i - BH * h) + 8], trp[:])

        # ---- phase G: final combine (w-part), then PE transpose to item rows ----
        trv = TRH[:].rearrange("p (i k) -> p i k", k=8)
        TRA, TRB = trv[:, :, 0:4], trv[:, :, 4:8]
        evb = EVT[:].rearrange("p (i o) -> p i o", o=1)[:, sl, :].broadcast_to([128, BH, R])
        wvb = KCMf[:].rearrange("p (i k) -> p i k", k=11)[:, sl, 9:10].broadcast_to([128, BH, R])
        og = sctH(BH * R)
        ogv = og[:].rearrange("p (i r) -> p i r", r=R)
        nc.vector.tensor_tensor(ogv, TRB, evb, op=OP.mult)
        nc.vector.tensor_tensor(ogv, TRA, ogv, op=OP.subtract)
        og2 = sctH(BH * R)
        nc.vector.tensor_tensor(og2[:].rearrange("p (i r) -> p i r", r=R),
                                wvb, SCR_[:].rearrange("p (i r) -> p i r", r=R), op=OP.mult)
        nc.vector.tensor_tensor(og[:], og[:], og2[:], op=OP.add)
        otp = pst(BH * R, 128, psb)
        ptrans(otp[:], og[:])
        nc.vector.tensor_copy(out_sbh[h][:], otp[:])
        nc.sync.dma_start(d["out"][BH * h:BH * (h + 1), :], out_sbh[h][:])


_NC_CACHE = {}


def build_nc():
    if "nc" in _NC_CACHE:
        return _NC_CACHE["nc"]
    nc = bacc.Bacc("TRN2", target_bir_lowering=False, debug=False)
    d = {}
    d["consts"] = nc.dram_tensor("consts", [128, 129], F32, kind="ExternalInput")
    d["xta"] = nc.dram_tensor("xta", [128, KIN // 128, BL], BF16, kind="ExternalInput")
    d["wta"] = nc.dram_tensor("wta", [KIN, DTOT], BF16, kind="ExternalInput")
    d["wtf"] = nc.dram_tensor("wtf", [128, KIN // 128, 6], BF16, kind="ExternalInput")
    d["memt"] = nc.dram_tensor("memt", [BL // 2, W, 2, M], BF16, kind="ExternalInput")
    d["memn"] = nc.dram_tensor("memn", [BL // 2, 128, 2, M // 128, W], BF16, kind="ExternalInput")
    d["llt"] = nc.dram_tensor("llt", [BL, 128, 2, M // 128, M], FP8, kind="ExternalInput")
    d["rwt"] = nc.dram_tensor("rwt", [128, M // 128, BL * R], F32, kind="ExternalInput")
    d["prct"] = nc.dram_tensor("prct", [128, M // 128, BL], F32, kind="ExternalInput")
    d["wwt"] = nc.dram_tensor("wwt", [128, M // 128, BL], F32, kind="ExternalInput")
    d["usgt"] = nc.dram_tensor("usgt", [128, M // 128, BL], F32, kind="ExternalInput")
    d["diagt"] = nc.dram_tensor("diagt", [128, M // 128, BL], F32, kind="ExternalInput")
    d["out"] = nc.dram_tensor("out", [BL, R * W], F32, kind="ExternalOutput")
    with tile.TileContext(nc) as tc:
        with ExitStack() as ctx:
            _emit(nc, tc, ctx, d)
    nc.compile()
    _NC_CACHE["nc"] = nc
    return nc


def make_in_maps(inputs):
    names = ["rk", "rs", "wk", "ws", "ev", "wv", "fg", "ag", "wg", "rm"]
    Wall = np.concatenate([np.asarray(inputs[f"W_{n}"]) for n in names], axis=0).astype(np.float32)
    ball = np.concatenate([np.asarray(inputs[f"b_{n}"]) for n in names], axis=0).astype(np.float32)
    wta = np.zeros((KIN, DTOT), np.float32)
    wta[:IN] = Wall.T
    wta[IN] = ball
    wtf = np.ascontiguousarray(wta[:, 901:907]).astype(ml_dtypes.bfloat16)
    wta = wta.astype(ml_dtypes.bfloat16)
    consts = np.zeros((128, 129), np.float32)
    consts[:, :128] = np.eye(128, dtype=np.float32)
    consts[:, 128] = 1.0

    x = np.asarray(inputs["x"], np.float32)
    mem = np.asarray(inputs["memory"], np.float32)
    link = np.asarray(inputs["link_matrix"], np.float32)[:, 0]
    prec = np.asarray(inputs["precedence"], np.float32)[:, 0]
    rw = np.asarray(inputs["read_weights"], np.float32)
    wwin = np.asarray(inputs["write_weights"], np.float32)[:, 0]
    usage = np.asarray(inputs["usage_vector"], np.float32)

    # host-side relayouts so every DMA is contiguous per partition row
    wtf = np.ascontiguousarray(wtf.reshape(9, 128, 6).transpose(1, 0, 2))

    def mpart(a):
        # [M, J] -> [128, M//128, J] (slot chunks on partitions)
        return np.ascontiguousarray(a.reshape(NCH, 128, -1).transpose(1, 0, 2))

    in_maps = []
    for cix in range(NCORES):
        sl = slice(cix * BL, (cix + 1) * BL)
        xta = np.zeros((KIN, BL), np.float32)
        xta[:IN] = x[sl].T
        xta[IN] = 1.0
        xta = np.ascontiguousarray(
            xta.astype(ml_dtypes.bfloat16).reshape(9, 128, BL).transpose(1, 0, 2))
        rws = rw[sl]
        lk = link[sl]
        llt = np.stack([lk, lk.transpose(0, 2, 1)], axis=1) * LSC
        llt = np.ascontiguousarray(
            llt.astype(ml_dtypes.float8_e4m3fn).reshape(BL, 2, NCH, 128, M)
            .transpose(0, 3, 1, 2, 4))
        diag = np.ascontiguousarray(np.diagonal(lk, axis1=1, axis2=2))
        memt = mem[sl].transpose(0, 2, 1).astype(ml_dtypes.bfloat16)
        memt = np.ascontiguousarray(memt.reshape(BL // 2, 2, W, M).transpose(0, 2, 1, 3))
        memn = mem[sl].astype(ml_dtypes.bfloat16)
        memn = np.ascontiguousarray(
            memn.reshape(BL // 2, 2, NCH, 128, W).transpose(0, 3, 1, 2, 4))
        in_maps.append({
            "consts": consts,
            "xta": xta,
            "wta": wta,
            "wtf": wtf,
            "memt": memt,
            "memn": memn,
            "llt": llt,
            "rwt": mpart(rws.transpose(2, 0, 1).reshape(M, BL * R)),
            "prct": mpart(prec[sl].T),
            "wwt": mpart(wwin[sl].T),
            "usgt": mpart(usage[sl].T),
            "diagt": mpart(diag.T),
        })
    return in_maps


def kernel(**inputs):
    nc = build_nc()
    in_maps = make_in_maps(inputs)
    res = run_bass_kernel_spmd(nc, in_maps, list(range(NCORES))).results
    out = np.concatenate([res[c]["out"].reshape(BL, R, W) for c in range(NCORES)], axis=0)
    return out.astype(np.float32)

